# revision 16
# baseline (speedup 1.0000x reference)
"""Trainium2 Bass kernel for nn_AttentionModel (patch-transformer + MSE loss).

Model (per batch element b of B=32):
    x[b] : [L=32768] --instance-norm--> patches [T=1024, PS=32]
    h    = patches @ W_proj + b_proj                  [T, 256]
    qkv  = h @ W_qkv + b_qkv ;  q,k,v = split(qkv)    [T, 256] each
    attn = softmax(causal(q k^T / 16))                [T, T]
    out  = (attn @ v) @ W_out + b_out                 [T, 256]
    pred = out @ W_head + b_head                      [T, PS]
    loss = mean((pred[:, :-1] - patches[:, 1:])**2)   scalar

Sharding: data-parallel over batch, 4 batch elements per core x 8 cores.
Each core computes a partial sum-of-squares; host combines.

Implementation notes:
  - All activations are kept transposed [feature, token]: weight matrices
    stored [in, out] serve directly as matmul lhsT, so no activation
    transposes are needed outside of attention probabilities.
  - The patch-embed and qkv projections are algebraically fused on the host
    (W_eff = W_proj @ W_qkv), as are the output and head projections
    (W_oh = W_out @ W_head); this removes two full matmul stages.
  - Instance-norm is computed from the contiguous [128, 256] view of x; the
    per-series (x - mean) / std is applied during the transpose-epilogue
    copy that assembles the patch-transposed XnT [32, 1024] tile.
  - softmax is computed without the max-subtraction pass: scores are
    q.k/16 with q,k ~ N(0,1), |score| < ~40 in the worst case, so exp()
    cannot overflow fp32; omitting the row-max halves the softmax work.
  - Matmuls run in fp32r (full PE rate at free-dim>=256); the attention
    probabilities and v are cast to bf16 for the transposes and the PV
    matmul (p in [0,1], errors average out in the final mean).
"""

import math
import os

import numpy as np

import concourse.bass as bass
import concourse.mybir as mybir
import concourse.tile as tile
from concourse.bass_utils import run_bass_kernel_spmd
from concourse.masks import make_identity, make_lower_triangular
from concourse.vector_clock import ScopedClock

F32 = mybir.dt.float32
F32R = mybir.dt.float32r
BF16 = mybir.dt.bfloat16
AX = mybir.AxisListType
ALU = mybir.AluOpType
AF = mybir.ActivationFunctionType

N_CORES = 8
B = 32
L = 32768
PS = 32
D = 256
T = L // PS  # 1024
BPC = B // N_CORES  # batch elements per core = 4
NT = T // 128  # 8 t-tiles
SCALE = 1.0 / math.sqrt(D)  # 1/16


class SplitDrainTileContext(tile.TileContext):
    """TileContext whose final drain splits sem waits across multiple drain
    instructions -- this walrus rejects >1 sync wait per CTRL instruction."""

    def _drain_and_barrier(self, tick_clock, wait_clock):
        probe = mybir.InstDrain(name=f"I-{self.nc.next_id()}", ins=[], outs=[])
        probe.engine = mybir.EngineType.SP
        wait_clock.add_sem_waits(probe, ScopedClock({None: tick_clock.global_clock}))
        waits = list(probe.sync_info.on_wait) if probe.sync_info else []
        assert self.sems is not None
        handles = {h.num: h for h in self.sems.allocated().values()}
        if not waits:
            self.nc.sync.drain()
        for w in waits:
            d = self.nc.sync.drain()
            d.wait_op(handles[w.id], w.wait_value, "sem-ge", check=False)
        self.nc.all_engine_barrier()
        popped = self.nc._tile_sem_poison_stack.pop()
        assert popped is self._sem_poison
        self.nc.clear_and_free_semaphores(list(self.sems.allocated().values()))
        self.nc.all_engine_barrier()


def r32(ap):
    return ap.bitcast(F32R)


def split_excess_waits(nc, max_waits=1):
    """This walrus rejects instructions carrying more than one sync wait.
    Move extra waits onto no-op instructions inserted just before, on the
    same engine (sequential waits on one engine are equivalent to one
    multi-wait)."""
    for f in nc.m.functions:
        for blk in f.blocks:
            insts = list(blk.instructions)
            out = []
            changed = False
            for inst in insts:
                si = inst.sync_info
                waits = list(si.on_wait) if si else []
                if len(waits) > max_waits:
                    changed = True
                    extra, keep = waits[:-max_waits], waits[-max_waits:]
                    for w in extra:
                        nop = mybir.InstDrain(
                            name=f"I-{nc.next_id()}", ins=[], outs=[]
                        )
                        nop.engine = inst.engine
                        nop.sync_info = mybir.SyncInfo(on_wait=[w], on_update=[])
                        out.append(nop)
                    inst.sync_info = mybir.SyncInfo(
                        on_wait=keep, on_update=list(si.on_update)
                    )
                out.append(inst)
            if changed:
                blk.instructions = out


def build_program():
    nc = bass.Bass("TRN2", target_bir_lowering=False, debug=False, num_devices=N_CORES)

    x_d = nc.dram_tensor("x", [BPC, L], F32, kind="ExternalInput")
    weff_d = nc.dram_tensor("w_eff", [PS, 3 * D], F32R, kind="ExternalInput")
    bqk_d = nc.dram_tensor("b_qk", [128, 4], F32, kind="ExternalInput")
    bv_d = nc.dram_tensor("b_v", [1, D], F32, kind="ExternalInput")
    woh_d = nc.dram_tensor("w_oh", [128, 2 * PS], F32R, kind="ExternalInput")
    boh_d = nc.dram_tensor("b_oh", [PS, 1], F32, kind="ExternalInput")
    out_d = nc.dram_tensor("loss_partial", [1, 1], F32, kind="ExternalOutput")

    from contextlib import ExitStack

    with SplitDrainTileContext(nc) as tc, ExitStack() as ctx:
        cpool = ctx.enter_context(tc.tile_pool(name="consts", bufs=1))
        ppool_s = ctx.enter_context(tc.tile_pool(name="psum_s", bufs=3, space="PSUM"))
        ppool_o = ctx.enter_context(tc.tile_pool(name="psum_o", bufs=3, space="PSUM"))
        ppool_t = ctx.enter_context(tc.tile_pool(name="psum_t", bufs=2, space="PSUM"))
        xpool = ctx.enter_context(tc.tile_pool(name="xc", bufs=2))
        spool = ctx.enter_context(tc.tile_pool(name="small", bufs=4))
        bigpool = ctx.enter_context(tc.tile_pool(name="big", bufs=2))
        ppool = ctx.enter_context(tc.tile_pool(name="probs", bufs=6))
        ptpool = ctx.enter_context(tc.tile_pool(name="pt", bufs=3))
        scratch = ctx.enter_context(tc.tile_pool(name="scratch", bufs=3))

        if True:
            # ---- constants ----
            ident_f = cpool.tile([128, 128], F32)
            make_identity(nc, ident_f[:])
            ident_b = cpool.tile([128, 128], BF16)
            make_identity(nc, ident_b[:])
            tri_b = cpool.tile([128, 128], BF16)
            make_lower_triangular(nc, tri_b[:], val=1.0, diag=True)
            ones_col = cpool.tile([128, 1], F32)
            nc.vector.memset(ones_col[:], 1.0)
            ones_row = cpool.tile([1, 128], F32)
            nc.vector.memset(ones_row[:], 1.0)

            weff = cpool.tile([PS, 3 * D], F32R)
            nc.gpsimd.dma_start(weff[:], weff_d.ap()[:])
            bqk = cpool.tile([128, 4], F32)
            nc.gpsimd.dma_start(bqk[:], bqk_d.ap()[:])
            bv = cpool.tile([1, D], F32)
            nc.gpsimd.dma_start(bv[:], bv_d.ap()[:])
            woh = cpool.tile([128, 2 * PS], F32R)
            nc.gpsimd.dma_start(woh[:], woh_d.ap()[:])
            boh = cpool.tile([PS, 1], F32)
            nc.gpsimd.dma_start(boh[:], boh_d.ap()[:])

            # per-batch loss partials [32, BPC]
            lp_all = cpool.tile([PS, BPC], F32)
            nc.vector.memset(lp_all[:], 0.0)
            STAGE = int(os.environ.get("KSTAGE", "99"))

            for b in range(BPC):
                # ---- A: load x[b] contiguously as [128, 256] ----
                xc = xpool.tile([128, L // 128], F32)  # [128, 256]
                nc.gpsimd.dma_start(xc[:], x_d.ap()[b].rearrange("(p f) -> p f", p=128))

                if STAGE < 1:
                    continue
                # ---- B: instance-norm stats ----
                sums = spool.tile([128, 2], F32)
                nc.vector.reduce_sum(sums[:, 0:1], xc[:], axis=AX.X)
                sq_scr = scratch.tile([128, L // 128], F32)
                nc.scalar.activation(
                    sq_scr[:], xc[:], AF.Square, accum_out=sums[:, 1:2]
                )
                tot_ps = ppool_s.tile([1, 2], F32, tag="s")
                nc.tensor.matmul(tot_ps[:], ones_col[:], sums[:], start=True, stop=True)
                tot = spool.tile([1, 2], F32)
                nc.vector.tensor_copy(tot[:], tot_ps[:])

                # scalar chain: sc = [mean, sum*mean, ssq-s*m, std, std+eps, rstd, -m*rstd, -mean]
                sc = spool.tile([1, 8], F32)
                nc.scalar.mul(sc[:, 0:1], tot[:, 0:1], 1.0 / L)  # mean
                nc.vector.tensor_tensor(
                    out=sc[:, 1:2], in0=tot[:, 0:1], in1=sc[:, 0:1], op=ALU.mult
                )
                nc.vector.tensor_tensor(
                    out=sc[:, 2:3], in0=tot[:, 1:2], in1=sc[:, 1:2], op=ALU.subtract
                )
                nc.scalar.activation(
                    sc[:, 3:4], sc[:, 2:3], AF.Sqrt, scale=1.0 / (L - 1)
                )  # std (ddof=1)
                nc.vector.tensor_scalar_add(sc[:, 4:5], sc[:, 3:4], 1e-5)
                nc.vector.reciprocal(sc[:, 5:6], sc[:, 4:5])  # rstd
                nc.scalar.mul(sc[:, 7:8], sc[:, 0:1], -1.0)  # -mean
                nc.vector.tensor_tensor(
                    out=sc[:, 6:7], in0=sc[:, 7:8], in1=sc[:, 5:6], op=ALU.mult
                )  # -mean*rstd

                # broadcast [rstd, -mean*rstd] to 32 partitions via rank-1 matmul
                bc_ps = ppool_s.tile([PS, 2], F32, tag="s")
                nc.tensor.matmul(
                    bc_ps[:], ones_row[:, 0:PS], sc[:, 5:7], start=True, stop=True
                )
                bc = spool.tile([PS, 2], F32)
                nc.vector.tensor_copy(bc[:], bc_ps[:])

                if STAGE < 2:
                    continue
                # ---- C: transpose x into patch-major XnT [32, 1024], normalized ----
                xnt = bigpool.tile([PS, T], F32R, tag="xnt")
                for r in range(2):
                    xt_ps = ppool_t.tile([PS, 512], F32, tag="pt")
                    for c in range(4):
                        cc = 4 * r + c
                        nc.tensor.transpose(
                            xt_ps[:, c * 128 : (c + 1) * 128],
                            xc[:, cc * PS : (cc + 1) * PS],
                            ident_f[:],
                        )
                    # xt_ps[ps, c*128+i] holds patch t = 8*i + 4*r + c elem ps;
                    # write normalized values into xnt at those token columns.
                    nc.vector.tensor_scalar(
                        out=xnt.rearrange("p (i e) -> p e i", e=8)[
                            :, 4 * r : 4 * r + 4, :
                        ],
                        in0=xt_ps[:].rearrange("p (c i) -> p c i", c=4),
                        scalar1=bc[:, 0:1],
                        scalar2=bc[:, 1:2],
                        op0=ALU.mult,
                        op1=ALU.add,
                    )

                if STAGE < 3:
                    continue
                # ---- D: q^T, k^T = W_eff[:, :512]^T @ XnT   [4][128, 1024] ----
                qkt = bigpool.tile([128, 4 * T], F32R, tag="qkt")
                for m in range(4):
                    for n in range(2):
                        qk_ps = ppool_s.tile([128, 512], F32, tag="s")
                        nc.tensor.matmul(
                            qk_ps[:],
                            r32(weff[:, m * 128 : (m + 1) * 128]),
                            r32(xnt[:, n * 512 : (n + 1) * 512]),
                            start=True,
                            stop=True,
                        )
                        nc.scalar.activation(
                            qkt[:, m * T + n * 512 : m * T + (n + 1) * 512],
                            qk_ps[:],
                            AF.Identity,
                            bias=bqk[:, m : m + 1],
                        )

                if STAGE < 4:
                    continue
                # ---- E: v = XnT^T @ W_eff[:, 512:768] + b_v   [8][128, 256] bf16
                v_all = bigpool.tile([128, NT * D], BF16, tag="v")
                for j in range(NT):
                    v_ps = ppool_s.tile([128, D], F32, tag="s")
                    nc.tensor.matmul(
                        v_ps[:],
                        r32(xnt[:, j * 128 : (j + 1) * 128]),
                        r32(weff[:, 2 * D : 3 * D]),
                        start=True,
                        stop=False,
                    )
                    nc.tensor.matmul(
                        v_ps[:],
                        ones_row[:, 0:128].bitcast(F32),
                        bv[:],
                        start=False,
                        stop=True,
                    )
                    nc.any.tensor_copy(v_all[:, j * D : (j + 1) * D], v_ps[:])

                if STAGE < 5:
                    continue
                # ---- F: causal attention, two t-super-tiles of 512 ----
                ont = bigpool.tile([128, 2 * T], F32R, tag="ont")  # [dk][128, 1024]
                rr_all = spool.tile([128, NT], F32, tag="rr")
                for a in range(2):
                    o_ps = [ppool_o.tile([128, 512], F32, tag="o", name=f"o_{a}_{dk2}") for dk2 in range(2)]
                    p_sb = {}
                    for g in range(4 * a, 4 * a + 4):
                        ext = (g + 1) * 128
                        p_g = ppool.tile([128, 1024], BF16)
                        p_sb[g] = p_g
                        rs = spool.tile([128, 4], F32, tag="rs")
                        nparts = 0
                        nchunks = (ext + 511) // 512
                        for c in range(nchunks):
                            w = min(512, ext - c * 512)
                            s_ps = ppool_s.tile([128, 512], F32, tag="s")
                            for kd in range(2):
                                nc.tensor.matmul(
                                    s_ps[:, 0:w],
                                    r32(qkt[:, kd * T + g * 128 : kd * T + (g + 1) * 128]),
                                    r32(qkt[:, (2 + kd) * T + c * 512 : (2 + kd) * T + c * 512 + w]),
                                    start=(kd == 0),
                                    stop=(kd == 1),
                                )
                            # exp of sub-diagonal columns (with row-sum accum)
                            FS = int(os.environ.get("KFS", "99"))
                            wb = min(w, g * 128 - c * 512)
                            if FS >= 2 and wb > 0:
                                nc.scalar.activation(
                                    p_g[:, c * 512 : c * 512 + wb],
                                    s_ps[:, 0:wb],
                                    AF.Exp,
                                    scale=SCALE,
                                    accum_out=rs[:, nparts : nparts + 1],
                                )
                                nparts += 1
                            if FS >= 3 and c == nchunks - 1:
                                # diagonal block: exp, mask, accumulate row-sum
                                doff = g * 128 - c * 512
                                pd = scratch.tile([128, 128], BF16, tag="pd")
                                nc.scalar.activation(
                                    pd[:], s_ps[:, doff : doff + 128], AF.Exp,
                                    scale=SCALE,
                                )
                                nc.vector.tensor_tensor(
                                    out=p_g[:, g * 128 : (g + 1) * 128],
                                    in0=pd[:],
                                    in1=tri_b[:],
                                    op=ALU.mult,
                                )
                                nc.vector.reduce_sum(
                                    rs[:, nparts : nparts + 1],
                                    p_g[:, g * 128 : (g + 1) * 128],
                                    axis=AX.X,
                                )
                                nparts += 1
                        # total row-sum -> reciprocal -> normalize p
                        FS = int(os.environ.get("KFS", "99"))
                        if FS < 4:
                            continue
                        if nparts > 1:
                            nc.vector.reduce_sum(
                                rs[:, 3:4], rs[:, 0:nparts], axis=AX.X
                            )
                            rsum = rs[:, 3:4]
                        else:
                            rsum = rs[:, 0:1]
                        nc.vector.reciprocal(rr_all[:, g : g + 1], rsum)
                        nc.vector.tensor_scalar_mul(
                            p_g[:, 0:ext], p_g[:, 0:ext], rr_all[:, g : g + 1]
                        )

                    # transposes + PV accumulation
                    FS = int(os.environ.get("KFS", "99"))
                    if FS < 5:
                        continue
                    for j in range(4 * a + 4):
                        gmin = max(4 * a, j)
                        off = (gmin - 4 * a) * 128
                        pt_ps = ppool_t.tile([128, 512], BF16, tag="pt")
                        for g in range(gmin, 4 * a + 4):
                            go = (g - 4 * a) * 128
                            nc.tensor.transpose(
                                pt_ps[:, go : go + 128],
                                p_sb[g][:, j * 128 : (j + 1) * 128],
                                ident_b[:],
                            )
                        pt_sb = ptpool.tile([128, 512], BF16)
                        nc.any.tensor_copy(pt_sb[:, off:512], pt_ps[:, off:512])
                        if FS < 6:
                            continue
                        for dk in range(2):
                            nc.tensor.matmul(
                                o_ps[dk][:, off:512],
                                v_all[:, j * D + dk * 128 : j * D + dk * 128 + 128],
                                pt_sb[:, off:512],
                                start=(j == 0),
                                stop=(j == 4 * a + 3),
                            )
                    if FS >= 6:
                        for dk in range(2):
                            nc.any.tensor_copy(
                                ont[:, dk * T + a * 512 : dk * T + (a + 1) * 512],
                                o_ps[dk][:],
                            )

                if STAGE < 6:
                    continue
                # ---- G: pred^T = W_oh^T @ OnT + b_oh   [32, 1024] ----
                predt = bigpool.tile([PS, T], F32, tag="pred")
                for n in range(2):
                    pr_ps = ppool_s.tile([PS, 512], F32, tag="s")
                    for kd in range(2):
                        nc.tensor.matmul(
                            pr_ps[:],
                            r32(woh[:, kd * PS : (kd + 1) * PS]),
                            r32(ont[:, kd * T + n * 512 : kd * T + (n + 1) * 512]),
                            start=(kd == 0),
                            stop=(kd == 1),
                        )
                    nc.scalar.activation(
                        predt[:, n * 512 : (n + 1) * 512],
                        pr_ps[:],
                        AF.Identity,
                        bias=boh[:],
                    )

                if STAGE < 7:
                    continue
                # ---- H: loss partial: sum((pred[:, :-1] - patches[:, 1:])^2)
                dd = scratch.tile([PS, T], F32, tag="dd")
                nc.vector.tensor_tensor(
                    out=dd[:, 0 : T - 1],
                    in0=predt[:, 0 : T - 1],
                    in1=xnt[:, 1:T].bitcast(F32),
                    op=ALU.subtract,
                )
                nc.scalar.activation(
                    dd[:, 0 : T - 1],
                    dd[:, 0 : T - 1],
                    AF.Square,
                    accum_out=lp_all[:, b : b + 1],
                )

            # ---- final: total partial over batches & partitions ----
            lsum = spool.tile([PS, 1], F32)
            nc.vector.reduce_sum(lsum[:], lp_all[:], axis=AX.X)
            tot_ps = ppool_s.tile([1, 1], F32, tag="s")
            nc.tensor.matmul(
                tot_ps[:], ones_col[0:PS, :], lsum[:], start=True, stop=True
            )
            out_sb = spool.tile([1, 1], F32)
            nc.vector.tensor_copy(out_sb[:], tot_ps[:])
            nc.gpsimd.dma_start(out_d.ap()[:], out_sb[:])

    split_excess_waits(nc)
    return nc


_program_cache = {}


def _get_program():
    if "nc" not in _program_cache:
        _program_cache["nc"] = build_program()
    return _program_cache["nc"]


def make_in_maps(x, W_proj, b_proj, W_qkv, b_qkv, W_out, b_out, W_head, b_head):
    w_eff = (W_proj.astype(np.float64) @ W_qkv.astype(np.float64)).astype(np.float32)
    b_eff = (
        b_proj.astype(np.float64) @ W_qkv.astype(np.float64) + b_qkv.astype(np.float64)
    ).astype(np.float32)
    w_oh = (W_out.astype(np.float64) @ W_head.astype(np.float64)).astype(np.float32)
    b_oh = (
        b_out.astype(np.float64) @ W_head.astype(np.float64) + b_head.astype(np.float64)
    ).astype(np.float32)

    b_qk = np.ascontiguousarray(b_eff[: 2 * D].reshape(4, 128).T)  # [128, 4]
    b_v = np.ascontiguousarray(b_eff[2 * D :].reshape(1, D))  # [1, 256]
    # w_oh packed [128, 64]: col kd*32+j = W_oh[kd*128+p, j]
    woh_packed = np.concatenate([w_oh[0:128, :], w_oh[128:256, :]], axis=1)
    woh_packed = np.ascontiguousarray(woh_packed)
    boh_col = np.ascontiguousarray(b_oh.reshape(PS, 1))

    in_maps = []
    for core in range(N_CORES):
        xs = np.ascontiguousarray(x[core * BPC : (core + 1) * BPC])
        in_maps.append(
            {
                "x": xs,
                "w_eff": w_eff,
                "b_qk": b_qk,
                "b_v": b_v,
                "w_oh": woh_packed,
                "b_oh": boh_col,
            }
        )
    return in_maps


def kernel(**inputs) -> np.ndarray:
    inputs = {k: np.asarray(v) for k, v in inputs.items()}
    nc = _get_program()
    in_maps = make_in_maps(**inputs)
    res = run_bass_kernel_spmd(nc, in_maps, core_ids=list(range(N_CORES)))
    total = sum(float(res.results[i]["loss_partial"][0, 0]) for i in range(N_CORES))
    loss = total / (B * (T - 1) * PS)
    return np.float32(loss)


if __name__ == "__main__":
    # quick self-driven smoke (random weights, compare against numpy ref)
    rng = np.random.default_rng(0)
    ins = {
        "x": rng.standard_normal((B, L)).astype(np.float32),
        "W_proj": (rng.standard_normal((PS, D)) / math.sqrt(PS)).astype(np.float32),
        "b_proj": np.zeros(D, np.float32),
        "W_qkv": (rng.standard_normal((D, 3 * D)) / math.sqrt(D)).astype(np.float32),
        "b_qkv": np.zeros(3 * D, np.float32),
        "W_out": (rng.standard_normal((D, D)) / math.sqrt(D)).astype(np.float32),
        "b_out": np.zeros(D, np.float32),
        "W_head": (rng.standard_normal((D, PS)) / math.sqrt(D)).astype(np.float32),
        "b_head": np.zeros(PS, np.float32),
    }
    got = kernel(**ins)
    print("kernel loss:", got)


# revision 19
# speedup vs baseline: 1.1853x; 1.1853x over previous
"""Trainium2 Bass kernel for nn_AttentionModel (patch-transformer + MSE loss).

Model (per batch element b of B=32):
    x[b] : [L=32768] --instance-norm--> patches [T=1024, PS=32]
    h    = patches @ W_proj + b_proj                  [T, 256]
    qkv  = h @ W_qkv + b_qkv ;  q,k,v = split(qkv)    [T, 256] each
    attn = softmax(causal(q k^T / 16))                [T, T]
    out  = (attn @ v) @ W_out + b_out                 [T, 256]
    pred = out @ W_head + b_head                      [T, PS]
    loss = mean((pred[:, :-1] - patches[:, 1:])**2)   scalar

Sharding: data-parallel over batch, 4 batch elements per core x 8 cores.
Each core computes a partial sum-of-squares; host combines.

Implementation notes:
  - All activations are kept transposed [feature, token]: weight matrices
    stored [in, out] serve directly as matmul lhsT, so no activation
    transposes are needed outside of attention probabilities.
  - The patch-embed and qkv projections are algebraically fused on the host
    (W_eff = W_proj @ W_qkv), as are the output and head projections
    (W_oh = W_out @ W_head); this removes two full matmul stages.
  - The qkv bias is folded in by augmenting XnT with a constant-1 row and
    W_eff with a bias row (K=32 -> 33); rank-1 bias matmuls measured
    ~430 ns each on HW, far more than the extra contraction row.
  - Instance-norm is computed from the contiguous [128, 256] view of x; the
    per-series (x - mean) / std is applied during the transpose-epilogue
    copy that assembles the patch-transposed XnT [33, 1024] tile.
  - softmax is computed without the max-subtraction pass: scores are
    q.k/16 with q,k ~ N(0,1), |score| < ~40 in the worst case, so exp()
    cannot overflow fp32; omitting the row-max halves the softmax work.
  - sqrt(var) is computed as exp(0.5*ln(var)) so every ScalarE function
    comes from the natural_log_exp table set -- a Sqrt would force a
    ~2.7us ACT table reload per batch iteration.
  - q, k, attention probabilities, and v are bf16 (fast PE weight loads;
    the per-element rounding averages out in the final mean); the att-
    ention-out/pred matmuls run in fp32r (full PE rate at free-dim>=256).
"""

import math
import os

import numpy as np

import concourse.bass as bass
import concourse.mybir as mybir
import concourse.tile as tile
from concourse.bass_utils import run_bass_kernel_spmd
from concourse.masks import make_identity, make_lower_triangular
from concourse.vector_clock import ScopedClock

F32 = mybir.dt.float32
F32R = mybir.dt.float32r
BF16 = mybir.dt.bfloat16
AX = mybir.AxisListType
ALU = mybir.AluOpType
AF = mybir.ActivationFunctionType

N_CORES = 8
B = 32
L = 32768
PS = 32
D = 256
T = L // PS  # 1024
BPC = B // N_CORES  # batch elements per core = 4
NT = T // 128  # 8 t-tiles
KA = PS + 1  # augmented contraction dim (extra ones row for bias)
SCALE = 1.0 / math.sqrt(D)  # 1/16


class SplitDrainTileContext(tile.TileContext):
    """TileContext whose final drain splits sem waits across multiple drain
    instructions -- this walrus rejects >1 sync wait per instruction."""

    def _drain_and_barrier(self, tick_clock, wait_clock):
        probe = mybir.InstDrain(name=f"I-{self.nc.next_id()}", ins=[], outs=[])
        probe.engine = mybir.EngineType.SP
        wait_clock.add_sem_waits(probe, ScopedClock({None: tick_clock.global_clock}))
        waits = list(probe.sync_info.on_wait) if probe.sync_info else []
        assert self.sems is not None
        handles = {h.num: h for h in self.sems.allocated().values()}
        if not waits:
            self.nc.sync.drain()
        for w in waits:
            d = self.nc.sync.drain()
            d.wait_op(handles[w.id], w.wait_value, "sem-ge", check=False)
        self.nc.all_engine_barrier()
        popped = self.nc._tile_sem_poison_stack.pop()
        assert popped is self._sem_poison
        self.nc.clear_and_free_semaphores(list(self.sems.allocated().values()))
        self.nc.all_engine_barrier()


def r32(ap):
    return ap.bitcast(F32R)


def split_excess_waits(nc, max_waits=1):
    """This walrus rejects instructions carrying more than one sync wait.
    Hoist extra waits onto the immediately preceding same-engine
    instruction when that instruction signals nothing (no on_update -- then
    waiting before it cannot deadlock anyone), else insert a wait-only
    drain just before."""
    for f in nc.m.functions:
        for blk in f.blocks:
            insts = list(blk.instructions)
            out = []
            prev_by_engine = {}
            changed = False
            for inst in insts:
                si = inst.sync_info
                waits = list(si.on_wait) if si else []
                if len(waits) > max_waits:
                    changed = True
                    extra, keep = waits[:-max_waits], waits[-max_waits:]
                    remaining = []
                    prev = prev_by_engine.get(str(inst.engine))
                    for w in extra:
                        psi = prev.sync_info if prev is not None else None
                        if prev is not None and (
                            psi is None
                            or (len(psi.on_wait) == 0 and len(psi.on_update) == 0)
                        ):
                            prev.sync_info = mybir.SyncInfo(
                                on_wait=[w], on_update=[]
                            )
                            prev = None  # one hoist per predecessor
                        else:
                            remaining.append(w)
                    for w in remaining:
                        dr = mybir.InstDrain(
                            name=f"I-{nc.next_id()}", ins=[], outs=[]
                        )
                        dr.engine = inst.engine
                        dr.sync_info = mybir.SyncInfo(on_wait=[w], on_update=[])
                        out.append(dr)
                    inst.sync_info = mybir.SyncInfo(
                        on_wait=keep, on_update=list(si.on_update)
                    )
                out.append(inst)
                prev_by_engine[str(inst.engine)] = inst
            if changed:
                blk.instructions = out


def build_program():
    nc = bass.Bass("TRN2", target_bir_lowering=False, debug=False, num_devices=N_CORES)

    x_d = nc.dram_tensor("x", [BPC, L], F32, kind="ExternalInput")
    # [33, 768]: rows 0..31 = W_proj @ W_qkv, row 32 = b_proj @ W_qkv + b_qkv
    weff_d = nc.dram_tensor("w_eff", [KA, 3 * D], BF16, kind="ExternalInput")
    weffv_d = nc.dram_tensor("w_eff_v", [KA, D], F32R, kind="ExternalInput")
    woh_d = nc.dram_tensor("w_oh", [128, 2 * PS], F32R, kind="ExternalInput")
    boh_d = nc.dram_tensor("b_oh", [PS, 1], F32, kind="ExternalInput")
    out_d = nc.dram_tensor("loss_partial", [1, 1], F32, kind="ExternalOutput")

    from contextlib import ExitStack

    with SplitDrainTileContext(nc) as tc, ExitStack() as ctx:
        cpool = ctx.enter_context(tc.tile_pool(name="consts", bufs=1))
        ppool_s = ctx.enter_context(tc.tile_pool(name="psum_s", bufs=3, space="PSUM"))
        ppool_o = ctx.enter_context(tc.tile_pool(name="psum_o", bufs=3, space="PSUM"))
        ppool_t = ctx.enter_context(tc.tile_pool(name="psum_t", bufs=2, space="PSUM"))
        xpool = ctx.enter_context(tc.tile_pool(name="xc", bufs=2))
        spool = ctx.enter_context(tc.tile_pool(name="small", bufs=4))
        bigpool = ctx.enter_context(tc.tile_pool(name="big", bufs=2))
        ppool = ctx.enter_context(tc.tile_pool(name="probs", bufs=6))
        ptpool = ctx.enter_context(tc.tile_pool(name="pt", bufs=3))
        scratch = ctx.enter_context(tc.tile_pool(name="scratch", bufs=3))

        # ---- constants ----
        ident_f = cpool.tile([128, 128], F32)
        make_identity(nc, ident_f[:])
        ident_b = cpool.tile([128, 128], BF16)
        make_identity(nc, ident_b[:])
        tri_b = cpool.tile([128, 128], BF16)
        make_lower_triangular(nc, tri_b[:], val=1.0, diag=True)
        ones_col = cpool.tile([128, 1], F32)
        nc.vector.memset(ones_col[:], 1.0)
        ones_row = cpool.tile([1, PS], F32)
        nc.vector.memset(ones_row[:], 1.0)

        weff = cpool.tile([KA, 3 * D], BF16)
        nc.gpsimd.dma_start(weff[:], weff_d.ap()[:])
        weffv = cpool.tile([KA, D], F32R)
        nc.gpsimd.dma_start(weffv[:], weffv_d.ap()[:])
        woh = cpool.tile([128, 2 * PS], F32R)
        nc.gpsimd.dma_start(woh[:], woh_d.ap()[:])
        boh = cpool.tile([PS, 1], F32)
        nc.gpsimd.dma_start(boh[:], boh_d.ap()[:])

        # per-batch loss partials [32, BPC]
        lp_all = cpool.tile([PS, BPC], F32)

        for b in range(BPC):
            # ---- A: load x[b] contiguously as [128, 256] ----
            xc = xpool.tile([128, L // 128], F32)  # [128, 256]
            nc.gpsimd.dma_start(xc[:], x_d.ap()[b].rearrange("(p f) -> p f", p=128))

            # ---- B: instance-norm stats ----
            sums = spool.tile([128, 2], F32)
            nc.vector.reduce_sum(sums[:, 0:1], xc[:], axis=AX.X)
            sq_scr = scratch.tile([128, L // 128], F32)
            nc.scalar.activation(sq_scr[:], xc[:], AF.Square, accum_out=sums[:, 1:2])
            tot_ps = ppool_s.tile([1, 2], F32, tag="s")
            nc.tensor.matmul(tot_ps[:], ones_col[:], sums[:], start=True, stop=True)
            tot = spool.tile([1, 2], F32)
            nc.vector.tensor_copy(tot[:], tot_ps[:])

            # scalar chain: sc = [mean, s*m, ssq-s*m, ln(var), std, std+eps, rstd, -m*rstd, -mean]
            sc = spool.tile([1, 9], F32)
            nc.scalar.mul(sc[:, 0:1], tot[:, 0:1], 1.0 / L)  # mean
            nc.vector.tensor_tensor(
                out=sc[:, 1:2], in0=tot[:, 0:1], in1=sc[:, 0:1], op=ALU.mult
            )
            nc.vector.tensor_tensor(
                out=sc[:, 2:3], in0=tot[:, 1:2], in1=sc[:, 1:2], op=ALU.subtract
            )
            # std = sqrt(var) via exp(0.5*ln(var)) -- keeps ACT on one table set
            nc.scalar.activation(sc[:, 3:4], sc[:, 2:3], AF.Ln, scale=1.0 / (L - 1))
            nc.scalar.activation(sc[:, 4:5], sc[:, 3:4], AF.Exp, scale=0.5)
            nc.vector.tensor_scalar_add(sc[:, 5:6], sc[:, 4:5], 1e-5)
            nc.vector.reciprocal(sc[:, 6:7], sc[:, 5:6])  # rstd
            nc.scalar.mul(sc[:, 8:9], sc[:, 0:1], -1.0)  # -mean
            nc.vector.tensor_tensor(
                out=sc[:, 7:8], in0=sc[:, 8:9], in1=sc[:, 6:7], op=ALU.mult
            )  # -mean*rstd

            # broadcast [rstd, -mean*rstd] to 32 partitions via rank-1 matmul
            bc_ps = ppool_s.tile([PS, 2], F32, tag="s")
            nc.tensor.matmul(
                bc_ps[:], ones_row[:], sc[:, 6:8], start=True, stop=True,
            )
            bc = spool.tile([PS, 2], F32)
            nc.vector.tensor_copy(bc[:], bc_ps[:])

            # ---- C: transpose x into patch-major XnT [33, 1024], normalized;
            # row 32 is constant 1 (bias row for the augmented projections) ----
            xnt = bigpool.tile([KA, T], F32R, tag="xnt")
            nc.gpsimd.memset(xnt[PS : PS + 1, :].bitcast(F32), 1.0)
            for r in range(2):
                xt_ps = ppool_t.tile([PS, 512], F32, tag="pt")
                for c in range(4):
                    cc = 4 * r + c
                    nc.tensor.transpose(
                        xt_ps[:, c * 128 : (c + 1) * 128],
                        xc[:, cc * PS : (cc + 1) * PS],
                        ident_f[:],
                    )
                # xt_ps[ps, c*128+i] holds patch t = 8*i + 4*r + c elem ps;
                # write normalized values into xnt at those token columns.
                nc.vector.tensor_scalar(
                    out=xnt[0:PS, :].rearrange("p (i e) -> p e i", e=8)[
                        :, 4 * r : 4 * r + 4, :
                    ],
                    in0=xt_ps[:].rearrange("p (c i) -> p c i", c=4),
                    scalar1=bc[:, 0:1],
                    scalar2=bc[:, 1:2],
                    op0=ALU.mult,
                    op1=ALU.add,
                )

            # bf16 copy of XnT for the q/k projection (bf16 weights => FWL)
            xnt_b = bigpool.tile([KA, T], BF16, tag="xntb")
            nc.any.tensor_copy(xnt_b[:], xnt[:].bitcast(F32))

            # ---- D: q^T, k^T = W_eff[:, :512]^T @ XnT_aug  [4][128, 1024] bf16
            qkt = bigpool.tile([128, 4 * T], BF16, tag="qkt")
            for m in range(4):
                for n in range(2):
                    qk_ps = ppool_s.tile([128, 512], F32, tag="s")
                    nc.tensor.matmul(
                        qk_ps[:],
                        weff[:, m * 128 : (m + 1) * 128],
                        xnt_b[:, n * 512 : (n + 1) * 512],
                        start=True,
                        stop=True,
                    )
                    nc.any.tensor_copy(
                        qkt[:, m * T + n * 512 : m * T + (n + 1) * 512], qk_ps[:]
                    )

            # ---- E: v = XnT_aug^T @ W_eff_v   [8][128, 256] bf16 ----
            v_all = bigpool.tile([128, NT * D], BF16, tag="v")
            for j in range(NT):
                v_ps = ppool_s.tile([128, D], F32, tag="s")
                nc.tensor.matmul(
                    v_ps[:],
                    xnt[:, j * 128 : (j + 1) * 128],
                    weffv[:],
                    start=True,
                    stop=True,
                )
                nc.any.tensor_copy(v_all[:, j * D : (j + 1) * D], v_ps[:])

            # ---- F: causal attention, two t-super-tiles of 512 ----
            ont = bigpool.tile([128, 2 * T], F32R, tag="ont")  # [dk][128, 1024]
            rr_all = spool.tile([128, NT], F32, tag="rr")
            for a in range(2):
                o_ps = [
                    ppool_o.tile([128, 512], F32, tag="o", name=f"o_{a}_{dk2}")
                    for dk2 in range(2)
                ]
                p_sb = {}
                for g in range(4 * a, 4 * a + 4):
                    ext = (g + 1) * 128
                    p_g = ppool.tile([128, 1024], BF16)
                    p_sb[g] = p_g
                    rs = spool.tile([128, 4], F32, tag="rs")
                    nparts = 0
                    nchunks = (ext + 511) // 512
                    for c in range(nchunks):
                        w = min(512, ext - c * 512)
                        s_ps = ppool_s.tile([128, 512], F32, tag="s")
                        for kd in range(2):
                            nc.tensor.matmul(
                                s_ps[:, 0:w],
                                qkt[:, kd * T + g * 128 : kd * T + (g + 1) * 128],
                                qkt[
                                    :,
                                    (2 + kd) * T + c * 512 : (2 + kd) * T
                                    + c * 512
                                    + w,
                                ],
                                start=(kd == 0),
                                stop=(kd == 1),
                            )
                        # exp of sub-diagonal columns (with row-sum accum)
                        wb = min(w, g * 128 - c * 512)
                        if wb > 0:
                            nc.scalar.activation(
                                p_g[:, c * 512 : c * 512 + wb],
                                s_ps[:, 0:wb],
                                AF.Exp,
                                scale=SCALE,
                                accum_out=rs[:, nparts : nparts + 1],
                            )
                            nparts += 1
                        if c == nchunks - 1:
                            # diagonal block: exp, mask, accumulate row-sum
                            doff = g * 128 - c * 512
                            pd = scratch.tile([128, 128], BF16, tag="pd")
                            nc.scalar.activation(
                                pd[:], s_ps[:, doff : doff + 128], AF.Exp,
                                scale=SCALE,
                            )
                            nc.vector.tensor_tensor(
                                out=p_g[:, g * 128 : (g + 1) * 128],
                                in0=pd[:],
                                in1=tri_b[:],
                                op=ALU.mult,
                            )
                            nc.vector.reduce_sum(
                                rs[:, nparts : nparts + 1],
                                p_g[:, g * 128 : (g + 1) * 128],
                                axis=AX.X,
                            )
                            nparts += 1
                    # total row-sum -> reciprocal -> normalize p
                    if nparts > 1:
                        nc.vector.reduce_sum(rs[:, 3:4], rs[:, 0:nparts], axis=AX.X)
                        rsum = rs[:, 3:4]
                    else:
                        rsum = rs[:, 0:1]
                    nc.vector.reciprocal(rr_all[:, g : g + 1], rsum)
                    nc.vector.tensor_scalar_mul(
                        p_g[:, 0:ext], p_g[:, 0:ext], rr_all[:, g : g + 1]
                    )

                # transposes + PV accumulation
                for j in range(4 * a + 4):
                    gmin = max(4 * a, j)
                    off = (gmin - 4 * a) * 128
                    pt_ps = ppool_t.tile([128, 512], BF16, tag="pt")
                    for g in range(gmin, 4 * a + 4):
                        go = (g - 4 * a) * 128
                        nc.tensor.transpose(
                            pt_ps[:, go : go + 128],
                            p_sb[g][:, j * 128 : (j + 1) * 128],
                            ident_b[:],
                        )
                    pt_sb = ptpool.tile([128, 512], BF16)
                    nc.any.tensor_copy(pt_sb[:, off:512], pt_ps[:, off:512])
                    for dk in range(2):
                        nc.tensor.matmul(
                            o_ps[dk][:, off:512],
                            v_all[:, j * D + dk * 128 : j * D + dk * 128 + 128],
                            pt_sb[:, off:512],
                            start=(j == 0),
                            stop=(j == 4 * a + 3),
                        )
                for dk in range(2):
                    nc.any.tensor_copy(
                        ont[:, dk * T + a * 512 : dk * T + (a + 1) * 512],
                        o_ps[dk][:],
                    )

            # ---- G: pred^T = W_oh^T @ OnT + b_oh   [32, 1024] ----
            predt = bigpool.tile([PS, T], F32, tag="pred")
            for n in range(2):
                pr_ps = ppool_s.tile([PS, 512], F32, tag="s")
                for kd in range(2):
                    nc.tensor.matmul(
                        pr_ps[:],
                        woh[:, kd * PS : (kd + 1) * PS],
                        ont[:, kd * T + n * 512 : kd * T + (n + 1) * 512],
                        start=(kd == 0),
                        stop=(kd == 1),
                    )
                nc.scalar.activation(
                    predt[:, n * 512 : (n + 1) * 512],
                    pr_ps[:],
                    AF.Identity,
                    bias=boh[:],
                )

            # ---- H: loss partial: sum((pred[:, :-1] - patches[:, 1:])^2) ----
            dd = scratch.tile([PS, T], F32, tag="dd")
            nc.vector.tensor_tensor(
                out=dd[:, 0 : T - 1],
                in0=predt[:, 0 : T - 1],
                in1=xnt[0:PS, 1:T].bitcast(F32),
                op=ALU.subtract,
            )
            nc.scalar.activation(
                dd[:, 0 : T - 1],
                dd[:, 0 : T - 1],
                AF.Square,
                accum_out=lp_all[:, b : b + 1],
            )

        # ---- final: total partial over batches & partitions ----
        lsum = spool.tile([PS, 1], F32)
        nc.vector.reduce_sum(lsum[:], lp_all[:], axis=AX.X)
        tot_ps2 = ppool_s.tile([1, 1], F32, tag="s")
        nc.tensor.matmul(tot_ps2[:], ones_col[0:PS, :], lsum[:], start=True, stop=True)
        out_sb = spool.tile([1, 1], F32)
        nc.vector.tensor_copy(out_sb[:], tot_ps2[:])
        nc.gpsimd.dma_start(out_d.ap()[:], out_sb[:])

    split_excess_waits(nc)
    return nc


_program_cache = {}


def _get_program():
    if "nc" not in _program_cache:
        _program_cache["nc"] = build_program()
    return _program_cache["nc"]


def make_in_maps(x, W_proj, b_proj, W_qkv, b_qkv, W_out, b_out, W_head, b_head):
    import ml_dtypes

    w_eff = (W_proj.astype(np.float64) @ W_qkv.astype(np.float64)).astype(np.float64)
    b_eff = b_proj.astype(np.float64) @ W_qkv.astype(np.float64) + b_qkv.astype(
        np.float64
    )
    w_eff_aug = np.concatenate([w_eff, b_eff[None, :]], axis=0)  # [33, 768]
    w_oh = (W_out.astype(np.float64) @ W_head.astype(np.float64)).astype(np.float32)
    b_oh = (
        b_out.astype(np.float64) @ W_head.astype(np.float64) + b_head.astype(np.float64)
    ).astype(np.float32)

    weff_b = np.ascontiguousarray(w_eff_aug.astype(ml_dtypes.bfloat16))  # [33, 768]
    weffv = np.ascontiguousarray(w_eff_aug[:, 2 * D :].astype(np.float32))  # [33, 256]
    woh_packed = np.ascontiguousarray(
        np.concatenate([w_oh[0:128, :], w_oh[128:256, :]], axis=1)
    )  # [128, 64]
    boh_col = np.ascontiguousarray(b_oh.reshape(PS, 1))

    in_maps = []
    for core in range(N_CORES):
        xs = np.ascontiguousarray(x[core * BPC : (core + 1) * BPC])
        in_maps.append(
            {
                "x": xs,
                "w_eff": weff_b,
                "w_eff_v": weffv,
                "w_oh": woh_packed,
                "b_oh": boh_col,
            }
        )
    return in_maps


def kernel(**inputs) -> np.ndarray:
    inputs = {k: np.asarray(v) for k, v in inputs.items()}
    nc = _get_program()
    in_maps = make_in_maps(**inputs)
    res = run_bass_kernel_spmd(nc, in_maps, core_ids=list(range(N_CORES)))
    total = sum(float(res.results[i]["loss_partial"][0, 0]) for i in range(N_CORES))
    loss = total / (B * (T - 1) * PS)
    return np.float32(loss)


if __name__ == "__main__":
    rng = np.random.default_rng(0)
    ins = {
        "x": rng.standard_normal((B, L)).astype(np.float32),
        "W_proj": (rng.standard_normal((PS, D)) / math.sqrt(PS)).astype(np.float32),
        "b_proj": np.zeros(D, np.float32),
        "W_qkv": (rng.standard_normal((D, 3 * D)) / math.sqrt(D)).astype(np.float32),
        "b_qkv": np.zeros(3 * D, np.float32),
        "W_out": (rng.standard_normal((D, D)) / math.sqrt(D)).astype(np.float32),
        "b_out": np.zeros(D, np.float32),
        "W_head": (rng.standard_normal((D, PS)) / math.sqrt(D)).astype(np.float32),
        "b_head": np.zeros(PS, np.float32),
    }
    got = kernel(**ins)
    print("kernel loss:", got)


# revision 22
# speedup vs baseline: 1.2534x; 1.0575x over previous
"""Trainium2 Bass kernel for nn_AttentionModel (patch-transformer + MSE loss).

Model (per batch element b of B=32):
    x[b] : [L=32768] --instance-norm--> patches [T=1024, PS=32]
    h    = patches @ W_proj + b_proj                  [T, 256]
    qkv  = h @ W_qkv + b_qkv ;  q,k,v = split(qkv)    [T, 256] each
    attn = softmax(causal(q k^T / 16))                [T, T]
    out  = (attn @ v) @ W_out + b_out                 [T, 256]
    pred = out @ W_head + b_head                      [T, PS]
    loss = mean((pred[:, :-1] - patches[:, 1:])**2)   scalar

Sharding: data-parallel over batch, 4 batch elements per core x 8 cores.
Each core computes a partial sum-of-squares; host combines.

Implementation notes:
  - All activations are kept transposed [feature, token]: weight matrices
    stored [in, out] serve directly as matmul lhsT, so no activation
    transposes are needed outside of attention probabilities.
  - The patch-embed and qkv projections are algebraically fused on the host
    (W_eff = W_proj @ W_qkv), as are the output and head projections
    (W_oh = W_out @ W_head); this removes two full matmul stages.
  - The qkv bias is folded in by augmenting XnT with a constant-1 row and
    W_eff with a bias row (K=32 -> 33); rank-1 bias matmuls measured
    ~430 ns each on HW, far more than the extra contraction row.
  - Instance-norm is computed from the contiguous [128, 256] view of x; the
    per-series (x - mean) / std is applied during the transpose-epilogue
    copy that assembles the patch-transposed XnT [33, 1024] tile.
  - softmax is computed without the max-subtraction pass: scores are
    q.k/16 with q,k ~ N(0,1), |score| < ~40 in the worst case, so exp()
    cannot overflow fp32; omitting the row-max halves the softmax work.
  - sqrt(var) is computed as exp(0.5*ln(var)) so every ScalarE function
    comes from the natural_log_exp table set -- a Sqrt would force a
    ~2.7us ACT table reload per batch iteration.
  - q, k, attention probabilities, and v are bf16 (fast PE weight loads;
    the per-element rounding averages out in the final mean); the att-
    ention-out/pred matmuls run in fp32r (full PE rate at free-dim>=256).
"""

import math
import os

import numpy as np

import concourse.bass as bass
import concourse.mybir as mybir
import concourse.tile as tile
from concourse.bass_utils import run_bass_kernel_spmd
from concourse.masks import make_identity, make_lower_triangular
from concourse.vector_clock import ScopedClock

F32 = mybir.dt.float32
F32R = mybir.dt.float32r
BF16 = mybir.dt.bfloat16
AX = mybir.AxisListType
ALU = mybir.AluOpType
AF = mybir.ActivationFunctionType

N_CORES = 8
B = 32
L = 32768
PS = 32
D = 256
T = L // PS  # 1024
BPC = B // N_CORES  # batch elements per core = 4
NT = T // 128  # 8 t-tiles
KA = PS + 1  # augmented contraction dim (extra ones row for bias)
SCALE = 1.0 / math.sqrt(D)  # 1/16


class SplitDrainTileContext(tile.TileContext):
    """TileContext whose final drain splits sem waits across multiple drain
    instructions -- this walrus rejects >1 sync wait per instruction."""

    def _drain_and_barrier(self, tick_clock, wait_clock):
        probe = mybir.InstDrain(name=f"I-{self.nc.next_id()}", ins=[], outs=[])
        probe.engine = mybir.EngineType.SP
        wait_clock.add_sem_waits(probe, ScopedClock({None: tick_clock.global_clock}))
        waits = list(probe.sync_info.on_wait) if probe.sync_info else []
        assert self.sems is not None
        handles = {h.num: h for h in self.sems.allocated().values()}
        if not waits:
            self.nc.sync.drain()
        for w in waits:
            d = self.nc.sync.drain()
            d.wait_op(handles[w.id], w.wait_value, "sem-ge", check=False)
        self.nc.all_engine_barrier()
        popped = self.nc._tile_sem_poison_stack.pop()
        assert popped is self._sem_poison
        self.nc.clear_and_free_semaphores(list(self.sems.allocated().values()))
        self.nc.all_engine_barrier()


def r32(ap):
    return ap.bitcast(F32R)


def split_excess_waits(nc, max_waits=1):
    """This walrus rejects instructions carrying more than one sync wait.
    Hoist extra waits onto the immediately preceding same-engine
    instruction when that instruction signals nothing (no on_update -- then
    waiting before it cannot deadlock anyone), else insert a wait-only
    drain just before."""
    for f in nc.m.functions:
        for blk in f.blocks:
            insts = list(blk.instructions)
            out = []
            prev_by_engine = {}
            changed = False
            for inst in insts:
                si = inst.sync_info
                waits = list(si.on_wait) if si else []
                if len(waits) > max_waits:
                    changed = True
                    extra, keep = waits[:-max_waits], waits[-max_waits:]
                    remaining = []
                    prev = prev_by_engine.get(str(inst.engine))
                    for w in extra:
                        psi = prev.sync_info if prev is not None else None
                        if prev is not None and (
                            psi is None
                            or (len(psi.on_wait) == 0 and len(psi.on_update) == 0)
                        ):
                            prev.sync_info = mybir.SyncInfo(
                                on_wait=[w], on_update=[]
                            )
                            prev = None  # one hoist per predecessor
                        else:
                            remaining.append(w)
                    for w in remaining:
                        dr = mybir.InstDrain(
                            name=f"I-{nc.next_id()}", ins=[], outs=[]
                        )
                        dr.engine = inst.engine
                        dr.sync_info = mybir.SyncInfo(on_wait=[w], on_update=[])
                        out.append(dr)
                    inst.sync_info = mybir.SyncInfo(
                        on_wait=keep, on_update=list(si.on_update)
                    )
                out.append(inst)
                prev_by_engine[str(inst.engine)] = inst
            if changed:
                blk.instructions = out


def build_program():
    nc = bass.Bass("TRN2", target_bir_lowering=False, debug=False, num_devices=N_CORES)

    x_d = nc.dram_tensor("x", [BPC, L], F32, kind="ExternalInput")
    # [33, 768]: rows 0..31 = W_proj @ W_qkv, row 32 = b_proj @ W_qkv + b_qkv
    weff_d = nc.dram_tensor("w_eff", [KA, 3 * D], BF16, kind="ExternalInput")
    woh_d = nc.dram_tensor("w_oh", [128, 2 * PS], BF16, kind="ExternalInput")
    boh_d = nc.dram_tensor("b_oh", [PS, 1], F32, kind="ExternalInput")
    out_d = nc.dram_tensor("loss_partial", [1, 1], F32, kind="ExternalOutput")

    from contextlib import ExitStack

    with SplitDrainTileContext(nc) as tc, ExitStack() as ctx:
        cpool = ctx.enter_context(tc.tile_pool(name="consts", bufs=1))
        ppool_s = ctx.enter_context(tc.tile_pool(name="psum_s", bufs=3, space="PSUM"))
        ppool_o = ctx.enter_context(tc.tile_pool(name="psum_o", bufs=3, space="PSUM"))
        ppool_t = ctx.enter_context(tc.tile_pool(name="psum_t", bufs=2, space="PSUM"))
        xpool = ctx.enter_context(tc.tile_pool(name="xc", bufs=2))
        spool = ctx.enter_context(tc.tile_pool(name="small", bufs=4))
        bigpool = ctx.enter_context(tc.tile_pool(name="big", bufs=2))
        ppool = ctx.enter_context(tc.tile_pool(name="probs", bufs=6))
        ptpool = ctx.enter_context(tc.tile_pool(name="pt", bufs=3))
        scratch = ctx.enter_context(tc.tile_pool(name="scratch", bufs=3))

        # ---- constants ----
        ident_f = cpool.tile([128, 128], F32)
        make_identity(nc, ident_f[:])
        ident_b = cpool.tile([128, 128], BF16)
        make_identity(nc, ident_b[:])
        tri_b = cpool.tile([128, 128], BF16)
        make_lower_triangular(nc, tri_b[:], val=1.0, diag=True)
        ones_col = cpool.tile([128, 1], F32)
        nc.vector.memset(ones_col[:], 1.0)
        ones_row = cpool.tile([1, PS], F32)
        nc.vector.memset(ones_row[:], 1.0)

        weff = cpool.tile([KA, 3 * D], BF16)
        nc.gpsimd.dma_start(weff[:], weff_d.ap()[:])
        woh = cpool.tile([128, 2 * PS], BF16)
        nc.gpsimd.dma_start(woh[:], woh_d.ap()[:])
        boh = cpool.tile([PS, 1], F32)
        nc.gpsimd.dma_start(boh[:], boh_d.ap()[:])

        # per-batch loss partials [32, BPC]
        lp_all = cpool.tile([PS, BPC], F32)

        for b in range(BPC):
            # ---- A: load x[b] contiguously as [128, 256] ----
            xc = xpool.tile([128, L // 128], F32)  # [128, 256]
            nc.gpsimd.dma_start(xc[:], x_d.ap()[b].rearrange("(p f) -> p f", p=128))

            # ---- B: instance-norm stats ----
            sums = spool.tile([128, 2], F32)
            nc.vector.reduce_sum(sums[:, 0:1], xc[:], axis=AX.X)
            sq_scr = scratch.tile([128, L // 128], F32)
            nc.scalar.activation(sq_scr[:], xc[:], AF.Square, accum_out=sums[:, 1:2])
            tot_ps = ppool_s.tile([1, 2], F32, tag="s")
            nc.tensor.matmul(tot_ps[:], ones_col[:], sums[:], start=True, stop=True)
            tot = spool.tile([1, 2], F32)
            nc.vector.tensor_copy(tot[:], tot_ps[:])

            # scalar chain: sc = [mean, s*m, ssq-s*m, ln(var), std, std+eps, rstd, -m*rstd, -mean]
            sc = spool.tile([1, 9], F32)
            nc.scalar.mul(sc[:, 0:1], tot[:, 0:1], 1.0 / L)  # mean
            nc.vector.tensor_tensor(
                out=sc[:, 1:2], in0=tot[:, 0:1], in1=sc[:, 0:1], op=ALU.mult
            )
            nc.vector.tensor_tensor(
                out=sc[:, 2:3], in0=tot[:, 1:2], in1=sc[:, 1:2], op=ALU.subtract
            )
            # std = sqrt(var) via exp(0.5*ln(var)) -- keeps ACT on one table set
            nc.scalar.activation(sc[:, 3:4], sc[:, 2:3], AF.Ln, scale=1.0 / (L - 1))
            nc.scalar.activation(sc[:, 4:5], sc[:, 3:4], AF.Exp, scale=0.5)
            nc.vector.tensor_scalar_add(sc[:, 5:6], sc[:, 4:5], 1e-5)
            nc.vector.reciprocal(sc[:, 6:7], sc[:, 5:6])  # rstd
            nc.scalar.mul(sc[:, 8:9], sc[:, 0:1], -1.0)  # -mean
            nc.vector.tensor_tensor(
                out=sc[:, 7:8], in0=sc[:, 8:9], in1=sc[:, 6:7], op=ALU.mult
            )  # -mean*rstd

            # broadcast [rstd, -mean*rstd] to 32 partitions via rank-1 matmul
            bc_ps = ppool_s.tile([PS, 2], F32, tag="s")
            nc.tensor.matmul(
                bc_ps[:], ones_row[:], sc[:, 6:8], start=True, stop=True,
            )
            bc = spool.tile([PS, 2], F32)
            nc.vector.tensor_copy(bc[:], bc_ps[:])

            # ---- C: transpose x into patch-major XnT [33, 1024], normalized;
            # row 32 is constant 1 (bias row for the augmented projections) ----
            xnt = bigpool.tile([KA, T], F32, tag="xnt")
            nc.gpsimd.memset(xnt[PS : PS + 1, :], 1.0)
            for r in range(2):
                xt_ps = ppool_t.tile([PS, 512], F32, tag="pt")
                for c in range(4):
                    cc = 4 * r + c
                    nc.tensor.transpose(
                        xt_ps[:, c * 128 : (c + 1) * 128],
                        xc[:, cc * PS : (cc + 1) * PS],
                        ident_f[:],
                    )
                # xt_ps[ps, c*128+i] holds patch t = 8*i + 4*r + c elem ps;
                # write normalized values into xnt at those token columns.
                nc.vector.tensor_scalar(
                    out=xnt[0:PS, :].rearrange("p (i e) -> p e i", e=8)[
                        :, 4 * r : 4 * r + 4, :
                    ],
                    in0=xt_ps[:].rearrange("p (c i) -> p c i", c=4),
                    scalar1=bc[:, 0:1],
                    scalar2=bc[:, 1:2],
                    op0=ALU.mult,
                    op1=ALU.add,
                )

            # bf16 copy of XnT for the q/k projection (bf16 weights => FWL)
            xnt_b = bigpool.tile([KA, T], BF16, tag="xntb")
            nc.any.tensor_copy(xnt_b[:], xnt[:])

            # ---- D: q^T, k^T, v^T = W_eff^T @ XnT_aug  [6][128, 1024] bf16
            qkt = bigpool.tile([128, 6 * T], BF16, tag="qkt")
            for m in range(6):
                for n in range(2):
                    qk_ps = ppool_s.tile([128, 512], F32, tag="s")
                    nc.tensor.matmul(
                        qk_ps[:],
                        weff[:, m * 128 : (m + 1) * 128],
                        xnt_b[:, n * 512 : (n + 1) * 512],
                        start=True,
                        stop=True,
                    )
                    nc.any.tensor_copy(
                        qkt[:, m * T + n * 512 : m * T + (n + 1) * 512], qk_ps[:]
                    )

            # ---- E: VW = V @ W_oh   [8][128, 32] bf16 (s-major) ----
            # Folding W_out@W_head into V lets the PV matmul produce pred
            # directly: pred^T = (V W_oh)^T P_norm^T.
            vw_all = bigpool.tile([128, NT * PS], BF16, tag="vw")
            for j in range(NT):
                vw_ps = ppool_s.tile([128, PS], F32, tag="s")
                for dk in range(2):
                    nc.tensor.matmul(
                        vw_ps[:],
                        qkt[:, (4 + dk) * T + j * 128 : (4 + dk) * T + (j + 1) * 128],
                        woh[:, dk * PS : (dk + 1) * PS],
                        start=(dk == 0),
                        stop=(dk == 1),
                    )
                nc.any.tensor_copy(vw_all[:, j * PS : (j + 1) * PS], vw_ps[:])

            # ---- F: causal attention; PV accumulates pred^T directly ----
            predt = bigpool.tile([PS, T], F32, tag="pred")
            rr_all = spool.tile([128, NT], F32, tag="rr")
            for a in range(2):
                pred_ps = ppool_o.tile([PS, 512], F32, tag="o")
                p_sb = {}
                for g in range(4 * a, 4 * a + 4):
                    ext = (g + 1) * 128
                    p_g = ppool.tile([128, 1024], BF16)
                    p_sb[g] = p_g
                    rs = spool.tile([128, 4], F32, tag="rs")
                    nparts = 0
                    nchunks = (ext + 511) // 512
                    for c in range(nchunks):
                        w = min(512, ext - c * 512)
                        s_ps = ppool_s.tile([128, 512], F32, tag="s")
                        for kd in range(2):
                            nc.tensor.matmul(
                                s_ps[:, 0:w],
                                qkt[:, kd * T + g * 128 : kd * T + (g + 1) * 128],
                                qkt[
                                    :,
                                    (2 + kd) * T + c * 512 : (2 + kd) * T
                                    + c * 512
                                    + w,
                                ],
                                start=(kd == 0),
                                stop=(kd == 1),
                            )
                        # exp of sub-diagonal columns (with row-sum accum)
                        wb = min(w, g * 128 - c * 512)
                        if wb > 0:
                            nc.scalar.activation(
                                p_g[:, c * 512 : c * 512 + wb],
                                s_ps[:, 0:wb],
                                AF.Exp,
                                scale=SCALE,
                                accum_out=rs[:, nparts : nparts + 1],
                            )
                            nparts += 1
                        if c == nchunks - 1:
                            # diagonal block: exp, mask, accumulate row-sum
                            doff = g * 128 - c * 512
                            pd = scratch.tile([128, 128], BF16, tag="pd")
                            nc.scalar.activation(
                                pd[:], s_ps[:, doff : doff + 128], AF.Exp,
                                scale=SCALE,
                            )
                            nc.vector.tensor_tensor(
                                out=p_g[:, g * 128 : (g + 1) * 128],
                                in0=pd[:],
                                in1=tri_b[:],
                                op=ALU.mult,
                            )
                            nc.vector.reduce_sum(
                                rs[:, nparts : nparts + 1],
                                p_g[:, g * 128 : (g + 1) * 128],
                                axis=AX.X,
                            )
                            nparts += 1
                    # total row-sum -> reciprocal -> normalize p
                    if nparts > 1:
                        nc.vector.reduce_sum(rs[:, 3:4], rs[:, 0:nparts], axis=AX.X)
                        rsum = rs[:, 3:4]
                    else:
                        rsum = rs[:, 0:1]
                    nc.vector.reciprocal(rr_all[:, g : g + 1], rsum)

                # diag(1/rowsum) per t-tile: a regular matmul with lhsT = p
                # block and rhs = diag both transposes the block and applies
                # the softmax normalization in one pass.
                diag_all = scratch.tile([128, 512], BF16, tag="dg")
                for g in range(4 * a, 4 * a + 4):
                    go = (g - 4 * a) * 128
                    nc.vector.tensor_scalar_mul(
                        diag_all[:, go : go + 128], ident_b[:],
                        rr_all[:, g : g + 1],
                    )
                # transposes + PV(pred) accumulation
                for j in range(4 * a + 4):
                    gmin = max(4 * a, j)
                    off = (gmin - 4 * a) * 128
                    pt_ps = ppool_t.tile([128, 512], F32, tag="pt")
                    for g in range(gmin, 4 * a + 4):
                        go = (g - 4 * a) * 128
                        nc.tensor.matmul(
                            pt_ps[:, go : go + 128],
                            p_sb[g][:, j * 128 : (j + 1) * 128],
                            diag_all[:, go : go + 128],
                            start=True,
                            stop=True,
                        )
                    pt_sb = ptpool.tile([128, 512], BF16)
                    nc.any.tensor_copy(pt_sb[:, off:512], pt_ps[:, off:512])
                    nc.tensor.matmul(
                        pred_ps[:, off:512],
                        vw_all[:, j * PS : (j + 1) * PS],
                        pt_sb[:, off:512],
                        start=(j == 0),
                        stop=(j == 4 * a + 3),
                    )
                nc.scalar.activation(
                    predt[:, a * 512 : (a + 1) * 512],
                    pred_ps[:],
                    AF.Identity,
                    bias=boh[:],
                )

            # ---- H: loss partial: sum((pred[:, :-1] - patches[:, 1:])^2) ----
            dd = scratch.tile([PS, T], F32, tag="dd")
            nc.vector.tensor_tensor(
                out=dd[:, 0 : T - 1],
                in0=predt[:, 0 : T - 1],
                in1=xnt[0:PS, 1:T],
                op=ALU.subtract,
            )
            nc.scalar.activation(
                dd[:, 0 : T - 1],
                dd[:, 0 : T - 1],
                AF.Square,
                accum_out=lp_all[:, b : b + 1],
            )

        # ---- final: total partial over batches & partitions ----
        lsum = spool.tile([PS, 1], F32)
        nc.vector.reduce_sum(lsum[:], lp_all[:], axis=AX.X)
        tot_ps2 = ppool_s.tile([1, 1], F32, tag="s")
        nc.tensor.matmul(tot_ps2[:], ones_col[0:PS, :], lsum[:], start=True, stop=True)
        out_sb = spool.tile([1, 1], F32)
        nc.vector.tensor_copy(out_sb[:], tot_ps2[:])
        nc.gpsimd.dma_start(out_d.ap()[:], out_sb[:])

    split_excess_waits(nc)
    return nc


_program_cache = {}


def _get_program():
    if "nc" not in _program_cache:
        _program_cache["nc"] = build_program()
    return _program_cache["nc"]


def make_in_maps(x, W_proj, b_proj, W_qkv, b_qkv, W_out, b_out, W_head, b_head):
    import ml_dtypes

    w_eff = (W_proj.astype(np.float64) @ W_qkv.astype(np.float64)).astype(np.float64)
    b_eff = b_proj.astype(np.float64) @ W_qkv.astype(np.float64) + b_qkv.astype(
        np.float64
    )
    w_eff_aug = np.concatenate([w_eff, b_eff[None, :]], axis=0)  # [33, 768]
    w_oh = (W_out.astype(np.float64) @ W_head.astype(np.float64)).astype(np.float32)
    b_oh = (
        b_out.astype(np.float64) @ W_head.astype(np.float64) + b_head.astype(np.float64)
    ).astype(np.float32)

    weff_b = np.ascontiguousarray(w_eff_aug.astype(ml_dtypes.bfloat16))  # [33, 768]
    woh_packed = np.ascontiguousarray(
        np.concatenate([w_oh[0:128, :], w_oh[128:256, :]], axis=1).astype(
            ml_dtypes.bfloat16
        )
    )  # [128, 64]
    boh_col = np.ascontiguousarray(b_oh.reshape(PS, 1))

    in_maps = []
    for core in range(N_CORES):
        xs = np.ascontiguousarray(x[core * BPC : (core + 1) * BPC])
        in_maps.append(
            {
                "x": xs,
                "w_eff": weff_b,
                "w_oh": woh_packed,
                "b_oh": boh_col,
            }
        )
    return in_maps


def kernel(**inputs) -> np.ndarray:
    inputs = {k: np.asarray(v) for k, v in inputs.items()}
    nc = _get_program()
    in_maps = make_in_maps(**inputs)
    res = run_bass_kernel_spmd(nc, in_maps, core_ids=list(range(N_CORES)))
    total = sum(float(res.results[i]["loss_partial"][0, 0]) for i in range(N_CORES))
    loss = total / (B * (T - 1) * PS)
    return np.float32(loss)


if __name__ == "__main__":
    rng = np.random.default_rng(0)
    ins = {
        "x": rng.standard_normal((B, L)).astype(np.float32),
        "W_proj": (rng.standard_normal((PS, D)) / math.sqrt(PS)).astype(np.float32),
        "b_proj": np.zeros(D, np.float32),
        "W_qkv": (rng.standard_normal((D, 3 * D)) / math.sqrt(D)).astype(np.float32),
        "b_qkv": np.zeros(3 * D, np.float32),
        "W_out": (rng.standard_normal((D, D)) / math.sqrt(D)).astype(np.float32),
        "b_out": np.zeros(D, np.float32),
        "W_head": (rng.standard_normal((D, PS)) / math.sqrt(D)).astype(np.float32),
        "b_head": np.zeros(PS, np.float32),
    }
    got = kernel(**ins)
    print("kernel loss:", got)


# revision 25
# speedup vs baseline: 1.5752x; 1.2568x over previous
"""Trainium2 Bass kernel for nn_AttentionModel (patch-transformer + MSE loss).

Model (per batch element b of B=32):
    x[b] : [L=32768] --instance-norm--> patches [T=1024, PS=32]
    h    = patches @ W_proj + b_proj                  [T, 256]
    qkv  = h @ W_qkv + b_qkv ;  q,k,v = split(qkv)    [T, 256] each
    attn = softmax(causal(q k^T / 16))                [T, T]
    out  = (attn @ v) @ W_out + b_out                 [T, 256]
    pred = out @ W_head + b_head                      [T, PS]
    loss = mean((pred[:, :-1] - patches[:, 1:])**2)   scalar

Sharding: data-parallel over batch, 4 batch elements per core x 8 cores.
Each core computes a partial sum-of-squares; host combines.

Key algebraic restructure (host-precomputed constants):
    Xa        = [patches^T ; ones]                [33, T] (normalized on-chip)
    M_qk      = Wq_eff Wk_eff^T                   [33, 33]
    M_vo_aug  = [Wv_eff (W_out W_head), e_ones]   [33, 33]
  where Wq_eff = [W_proj W_qkv_q ; b_q_eff] etc. Then:
    scores^T  = Xa^T M_qk Xa      computed as Xa-dot-(M_qk^T Xa), already
                in [s, t] layout, so no PE transposes of the attention
                probabilities are needed.
    VW_aug    = Xa^T M_vo_aug                     [T, 33]
    predu/css = VW_aug^T exp(scores^T/16)         [33, T]
                rows 0..31 = unnormalized pred^T; row 32 = softmax
                denominator (the e_ones column of M_vo_aug sums exp
                columns through the ones row of Xa).
    pred^T    = predu * (1/css) + b_oh
All attention matmuls are bf16 (errors average out in the final scalar
mean); instance-norm, the softmax exp, denominators, and the loss are fp32.
softmax skips the max-subtraction pass: scores/16 ~ N(0,1) and bounded by
|q||k|/16 < ~40, so exp cannot overflow fp32.
sqrt(var) is computed as exp(0.5*ln(var)) so all ScalarE functions come
from one ACT table set (a Sqrt would force ~2.7us table reloads).
"""

import math
import os

import numpy as np

import concourse.bass as bass
import concourse.mybir as mybir
import concourse.tile as tile
from concourse.bass_utils import run_bass_kernel_spmd
from concourse.masks import make_identity, make_upper_triangular
from concourse.vector_clock import ScopedClock

F32 = mybir.dt.float32
BF16 = mybir.dt.bfloat16
AX = mybir.AxisListType
ALU = mybir.AluOpType
AF = mybir.ActivationFunctionType

N_CORES = 8
B = 32
L = 32768
PS = 32
D = 256
T = L // PS  # 1024
BPC = B // N_CORES  # batch elements per core = 4
NT = T // 128  # 8 s-tiles
KA = PS + 1  # augmented contraction dim (extra ones row)
SCALE = 1.0 / math.sqrt(D)  # 1/16


class SplitDrainTileContext(tile.TileContext):
    """TileContext whose final drain splits sem waits across multiple drain
    instructions -- this walrus rejects >1 sync wait per instruction."""

    def _drain_and_barrier(self, tick_clock, wait_clock):
        probe = mybir.InstDrain(name=f"I-{self.nc.next_id()}", ins=[], outs=[])
        probe.engine = mybir.EngineType.SP
        wait_clock.add_sem_waits(probe, ScopedClock({None: tick_clock.global_clock}))
        waits = list(probe.sync_info.on_wait) if probe.sync_info else []
        assert self.sems is not None
        handles = {h.num: h for h in self.sems.allocated().values()}
        if not waits:
            self.nc.sync.drain()
        for w in waits:
            d = self.nc.sync.drain()
            d.wait_op(handles[w.id], w.wait_value, "sem-ge", check=False)
        self.nc.all_engine_barrier()
        popped = self.nc._tile_sem_poison_stack.pop()
        assert popped is self._sem_poison
        self.nc.clear_and_free_semaphores(list(self.sems.allocated().values()))
        self.nc.all_engine_barrier()


def split_excess_waits(nc, max_waits=1):
    """This walrus rejects instructions carrying more than one sync wait.
    Hoist extra waits onto the immediately preceding same-engine
    instruction when that instruction signals nothing (then waiting before
    it cannot starve anyone), else insert a wait-only drain."""
    for f in nc.m.functions:
        for blk in f.blocks:
            insts = list(blk.instructions)
            out = []
            prev_by_engine = {}
            changed = False
            for inst in insts:
                si = inst.sync_info
                waits = list(si.on_wait) if si else []
                if len(waits) > max_waits:
                    changed = True
                    extra, keep = waits[:-max_waits], waits[-max_waits:]
                    remaining = []
                    prev = prev_by_engine.get(str(inst.engine))
                    for w in extra:
                        psi = prev.sync_info if prev is not None else None
                        if prev is not None and (
                            psi is None
                            or (len(psi.on_wait) == 0 and len(psi.on_update) == 0)
                        ):
                            prev.sync_info = mybir.SyncInfo(on_wait=[w], on_update=[])
                            prev = None  # one hoist per predecessor
                        else:
                            remaining.append(w)
                    for w in remaining:
                        dr = mybir.InstDrain(name=f"I-{nc.next_id()}", ins=[], outs=[])
                        dr.engine = inst.engine
                        dr.sync_info = mybir.SyncInfo(on_wait=[w], on_update=[])
                        out.append(dr)
                    inst.sync_info = mybir.SyncInfo(
                        on_wait=keep, on_update=list(si.on_update)
                    )
                out.append(inst)
                prev_by_engine[str(inst.engine)] = inst
            if changed:
                blk.instructions = out


def build_program():
    nc = bass.Bass("TRN2", target_bir_lowering=False, debug=False, num_devices=N_CORES)

    x_d = nc.dram_tensor("x", [BPC, L], F32, kind="ExternalInput")
    mqk_d = nc.dram_tensor("m_qk", [KA, KA], BF16, kind="ExternalInput")
    mvo_d = nc.dram_tensor("m_vo", [KA, KA], BF16, kind="ExternalInput")
    out_d = nc.dram_tensor("loss_partial", [1, 1], F32, kind="ExternalOutput")

    from contextlib import ExitStack

    with SplitDrainTileContext(nc) as tc, ExitStack() as ctx:
        cpool = ctx.enter_context(tc.tile_pool(name="consts", bufs=1))
        ppool_s = ctx.enter_context(tc.tile_pool(name="psum_s", bufs=3, space="PSUM"))
        ppool_o = ctx.enter_context(tc.tile_pool(name="psum_o", bufs=3, space="PSUM"))
        ppool_t = ctx.enter_context(tc.tile_pool(name="psum_t", bufs=2, space="PSUM"))
        xpool = ctx.enter_context(tc.tile_pool(name="xc", bufs=2))
        spool = ctx.enter_context(tc.tile_pool(name="small", bufs=4))
        bigpool = ctx.enter_context(tc.tile_pool(name="big", bufs=2))
        epool = ctx.enter_context(tc.tile_pool(name="et", bufs=10))
        scratch = ctx.enter_context(tc.tile_pool(name="scratch", bufs=3))

        # ---- constants ----
        ident_f = cpool.tile([128, 128], F32)
        make_identity(nc, ident_f[:])
        triu_b = cpool.tile([128, 128], BF16)
        make_upper_triangular(nc, triu_b[:], val=1.0, diag=True)
        ones_col = cpool.tile([128, 1], F32)
        nc.vector.memset(ones_col[:], 1.0)
        ones_row = cpool.tile([1, PS], F32)
        nc.vector.memset(ones_row[:], 1.0)
        ones_row_b = cpool.tile([1, PS], BF16)
        nc.vector.memset(ones_row_b[:], 1.0)

        mqk = cpool.tile([KA, KA], BF16)
        nc.gpsimd.dma_start(mqk[:], mqk_d.ap()[:])
        mvo = cpool.tile([KA, KA], BF16)
        nc.gpsimd.dma_start(mvo[:], mvo_d.ap()[:])

        lp_all = cpool.tile([PS, BPC], F32)  # per-batch loss partials

        for b in range(BPC):
            # ---- A: load x[b] contiguously as [128, 256] ----
            xc = xpool.tile([128, L // 128], F32)
            nc.gpsimd.dma_start(xc[:], x_d.ap()[b].rearrange("(p f) -> p f", p=128))

            # ---- B: instance-norm stats ----
            sums = spool.tile([128, 2], F32)
            nc.vector.reduce_sum(sums[:, 0:1], xc[:], axis=AX.X)
            sq_scr = scratch.tile([128, L // 128], F32)
            nc.scalar.activation(sq_scr[:], xc[:], AF.Square, accum_out=sums[:, 1:2])
            tot_ps = ppool_s.tile([1, 2], F32, tag="s")
            nc.tensor.matmul(tot_ps[:], ones_col[:], sums[:], start=True, stop=True)
            tot = spool.tile([1, 2], F32)
            nc.vector.tensor_copy(tot[:], tot_ps[:])

            # sc = [mean, s*m, ssq-s*m, ln, std, std+eps, rstd, -m*rstd, -mean]
            sc = spool.tile([1, 9], F32)
            nc.scalar.mul(sc[:, 0:1], tot[:, 0:1], 1.0 / L)  # mean
            nc.vector.tensor_tensor(
                out=sc[:, 1:2], in0=tot[:, 0:1], in1=sc[:, 0:1], op=ALU.mult
            )
            nc.vector.tensor_tensor(
                out=sc[:, 2:3], in0=tot[:, 1:2], in1=sc[:, 1:2], op=ALU.subtract
            )
            nc.scalar.activation(sc[:, 3:4], sc[:, 2:3], AF.Ln, scale=1.0 / (L - 1))
            nc.scalar.activation(sc[:, 4:5], sc[:, 3:4], AF.Exp, scale=0.5)  # std
            nc.vector.tensor_scalar_add(sc[:, 5:6], sc[:, 4:5], 1e-5)
            nc.vector.reciprocal(sc[:, 6:7], sc[:, 5:6])  # rstd
            nc.scalar.mul(sc[:, 8:9], sc[:, 0:1], -1.0)  # -mean
            nc.vector.tensor_tensor(
                out=sc[:, 7:8], in0=sc[:, 8:9], in1=sc[:, 6:7], op=ALU.mult
            )  # -mean*rstd

            # broadcast [rstd, -mean*rstd] to 32 partitions via rank-1 matmul
            bc_ps = ppool_s.tile([PS, 2], F32, tag="s")
            nc.tensor.matmul(bc_ps[:], ones_row[:], sc[:, 6:8], start=True, stop=True)
            bc = spool.tile([PS, 2], F32)
            nc.vector.tensor_copy(bc[:], bc_ps[:])

            # ---- C: transpose x into patch-major Xa [33, 1024], normalized;
            # row 32 is the constant-1 augmentation row ----
            xnt = bigpool.tile([KA, T], F32, tag="xnt")
            nc.gpsimd.memset(xnt[PS : PS + 1, :], 1.0)
            for r in range(2):
                xt_ps = ppool_t.tile([PS, 512], F32, tag="pt")
                for c in range(4):
                    cc = 4 * r + c
                    nc.tensor.transpose(
                        xt_ps[:, c * 128 : (c + 1) * 128],
                        xc[:, cc * PS : (cc + 1) * PS],
                        ident_f[:],
                    )
                # xt_ps[ps, c*128+i] holds patch t = 8*i + 4*r + c elem ps
                nc.vector.tensor_scalar(
                    out=xnt[0:PS, :].rearrange("p (i e) -> p e i", e=8)[
                        :, 4 * r : 4 * r + 4, :
                    ],
                    in0=xt_ps[:].rearrange("p (c i) -> p c i", c=4),
                    scalar1=bc[:, 0:1],
                    scalar2=bc[:, 1:2],
                    op0=ALU.mult,
                    op1=ALU.add,
                )
            xnt_b = bigpool.tile([KA, T], BF16, tag="xntb")
            nc.any.tensor_copy(xnt_b[:], xnt[:])

            # ---- D: Y = M_qk^T Xa  [33, 1024] bf16 ----
            y_b = bigpool.tile([KA, T], BF16, tag="y")
            for n in range(2):
                y_ps = ppool_s.tile([KA, 512], F32, tag="s")
                nc.tensor.matmul(
                    y_ps[:],
                    mqk[:],
                    xnt_b[:, n * 512 : (n + 1) * 512],
                    start=True,
                    stop=True,
                )
                nc.any.tensor_copy(y_b[:, n * 512 : (n + 1) * 512], y_ps[:])

            # ---- E: VW_aug = Xa^T M_vo_aug  [8][128, 33] bf16 ----
            vw_all = bigpool.tile([128, NT * KA], BF16, tag="vw")
            for j in range(NT):
                vw_ps = ppool_s.tile([128, KA], F32, tag="s")
                nc.tensor.matmul(
                    vw_ps[:],
                    xnt_b[:, j * 128 : (j + 1) * 128],
                    mvo[:],
                    start=True,
                    stop=True,
                )
                nc.any.tensor_copy(vw_all[:, j * KA : (j + 1) * KA], vw_ps[:])

            # ---- F: eT[j] = masked exp(scores^T/16)  [s-tile j][128, j*128..T]
            et = {}
            for j in range(NT):
                e_j = epool.tile([128, T], BF16, tag="et", name=f"et_{b}_{j}")
                et[j] = e_j
                c0 = (j * 128) // 512
                for c in range(c0, 2):
                    off = max(0, j * 128 - c * 512)  # within-chunk start
                    sT_ps = ppool_s.tile([128, 512], F32, tag="s")
                    nc.tensor.matmul(
                        sT_ps[:, off:512],
                        xnt_b[:, j * 128 : (j + 1) * 128],
                        y_b[:, c * 512 + off : (c + 1) * 512],
                        start=True,
                        stop=True,
                    )
                    nc.scalar.activation(
                        e_j[:, c * 512 + off : (c + 1) * 512],
                        sT_ps[:, off:512],
                        AF.Exp,
                        scale=SCALE,
                    )
                    if c == c0:
                        # diagonal block: zero the s > t half (keep s <= t)
                        nc.vector.tensor_tensor(
                            out=e_j[:, j * 128 : (j + 1) * 128],
                            in0=e_j[:, j * 128 : (j + 1) * 128],
                            in1=triu_b[:],
                            op=ALU.mult,
                        )

            # ---- G: [pred_u ; colsum] = VW_aug^T eT, then normalize ----
            predt = bigpool.tile([PS, T], F32, tag="pred")
            for n in range(2):
                pu_ps = ppool_o.tile([KA, 512], F32, tag="o")
                for j in range(4 * n + 4):
                    off = max(0, j * 128 - n * 512)
                    nc.tensor.matmul(
                        pu_ps[:, off:512],
                        vw_all[:, j * KA : (j + 1) * KA],
                        et[j][:, n * 512 + off : (n + 1) * 512],
                        start=(j == 0),
                        stop=(j == 4 * n + 3),
                    )
                # pred = pred_u / colsum + b_oh
                rr = spool.tile([1, 512], F32, tag="rrow")
                nc.vector.reciprocal(rr[:], pu_ps[PS : PS + 1, :])
                rr_b = spool.tile([1, 512], BF16, tag="rrowb")
                nc.vector.tensor_copy(rr_b[:], rr[:])
                bcr_ps = ppool_t.tile([PS, 512], F32, tag="pt")
                nc.tensor.matmul(
                    bcr_ps[:], ones_row_b[:], rr_b[:], start=True, stop=True
                )
                bcr_sb = scratch.tile([PS, 512], F32, tag="pn")
                nc.any.tensor_copy(bcr_sb[:], bcr_ps[:])
                nc.vector.tensor_tensor(
                    out=predt[:, n * 512 : (n + 1) * 512],
                    in0=pu_ps[0:PS, :],
                    in1=bcr_sb[:],
                    op=ALU.mult,
                )

            # ---- H: loss partial: sum((pred[:, :-1] - patches[:, 1:])^2) ----
            dd = scratch.tile([PS, T], F32, tag="dd")
            nc.vector.tensor_tensor(
                out=dd[:, 0 : T - 1],
                in0=predt[:, 0 : T - 1],
                in1=xnt[0:PS, 1:T],
                op=ALU.subtract,
            )
            nc.scalar.activation(
                dd[:, 0 : T - 1],
                dd[:, 0 : T - 1],
                AF.Square,
                accum_out=lp_all[:, b : b + 1],
            )

        # ---- final: total partial over batches & partitions ----
        lsum = spool.tile([PS, 1], F32)
        nc.vector.reduce_sum(lsum[:], lp_all[:], axis=AX.X)
        tot_ps2 = ppool_s.tile([1, 1], F32, tag="s")
        nc.tensor.matmul(tot_ps2[:], ones_col[0:PS, :], lsum[:], start=True, stop=True)
        out_sb = spool.tile([1, 1], F32)
        nc.vector.tensor_copy(out_sb[:], tot_ps2[:])
        nc.gpsimd.dma_start(out_d.ap()[:], out_sb[:])

    split_excess_waits(nc)
    return nc


_program_cache = {}


def _get_program():
    if "nc" not in _program_cache:
        _program_cache["nc"] = build_program()
    return _program_cache["nc"]


def make_in_maps(x, W_proj, b_proj, W_qkv, b_qkv, W_out, b_out, W_head, b_head):
    import ml_dtypes

    f8 = np.float64
    w_eff = W_proj.astype(f8) @ W_qkv.astype(f8)  # [32, 768]
    b_eff = b_proj.astype(f8) @ W_qkv.astype(f8) + b_qkv.astype(f8)  # [768]
    w_aug = np.concatenate([w_eff, b_eff[None, :]], axis=0)  # [33, 768]
    wq, wk, wv = w_aug[:, 0:D], w_aug[:, D : 2 * D], w_aug[:, 2 * D : 3 * D]
    m_qk = wq @ wk.T  # [33, 33]
    w_oh = W_out.astype(f8) @ W_head.astype(f8)  # [256, 32]
    b_oh = b_out.astype(f8) @ W_head.astype(f8) + b_head.astype(f8)  # [32]
    m_vo = wv @ w_oh  # [33, 32]
    # folding b_oh into the bias row: pred_u' = sum_s (VW + b_oh) eT, so
    # pred_u'/colsum = pred + b_oh exactly.
    m_vo[PS, :] += b_oh
    e_ones = np.zeros((KA, 1), f8)
    e_ones[PS, 0] = 1.0  # selects Xa's ones row -> colsum output column
    m_vo_aug = np.concatenate([m_vo, e_ones], axis=1)  # [33, 33]

    mqk_b = np.ascontiguousarray(m_qk.astype(ml_dtypes.bfloat16))
    mvo_b = np.ascontiguousarray(m_vo_aug.astype(ml_dtypes.bfloat16))

    in_maps = []
    for core in range(N_CORES):
        xs = np.ascontiguousarray(x[core * BPC : (core + 1) * BPC])
        in_maps.append({"x": xs, "m_qk": mqk_b, "m_vo": mvo_b})
    return in_maps


def kernel(**inputs) -> np.ndarray:
    inputs = {k: np.asarray(v) for k, v in inputs.items()}
    nc = _get_program()
    in_maps = make_in_maps(**inputs)
    res = run_bass_kernel_spmd(nc, in_maps, core_ids=list(range(N_CORES)))
    total = sum(float(res.results[i]["loss_partial"][0, 0]) for i in range(N_CORES))
    loss = total / (B * (T - 1) * PS)
    return np.float32(loss)


if __name__ == "__main__":
    rng = np.random.default_rng(0)
    ins = {
        "x": rng.standard_normal((B, L)).astype(np.float32),
        "W_proj": (rng.standard_normal((PS, D)) / math.sqrt(PS)).astype(np.float32),
        "b_proj": np.zeros(D, np.float32),
        "W_qkv": (rng.standard_normal((D, 3 * D)) / math.sqrt(D)).astype(np.float32),
        "b_qkv": np.zeros(3 * D, np.float32),
        "W_out": (rng.standard_normal((D, D)) / math.sqrt(D)).astype(np.float32),
        "b_out": np.zeros(D, np.float32),
        "W_head": (rng.standard_normal((D, PS)) / math.sqrt(D)).astype(np.float32),
        "b_head": np.zeros(PS, np.float32),
    }
    got = kernel(**ins)
    print("kernel loss:", got)


# revision 26
# speedup vs baseline: 1.8582x; 1.1796x over previous
"""Trainium2 Bass kernel for nn_AttentionModel (patch-transformer + MSE loss).

Model (per batch element b of B=32):
    x[b] : [L=32768] --instance-norm--> patches [T=1024, PS=32]
    h    = patches @ W_proj + b_proj                  [T, 256]
    qkv  = h @ W_qkv + b_qkv ;  q,k,v = split(qkv)    [T, 256] each
    attn = softmax(causal(q k^T / 16))                [T, T]
    out  = (attn @ v) @ W_out + b_out                 [T, 256]
    pred = out @ W_head + b_head                      [T, PS]
    loss = mean((pred[:, :-1] - patches[:, 1:])**2)   scalar

Sharding: data-parallel over batch, 4 batch elements per core x 8 cores.
Each core computes a partial sum-of-squares; host combines.

Key algebraic restructure (host-precomputed constants):
    Xa        = [patches^T ; ones]                [33, T] (normalized on-chip)
    M_qk      = Wq_eff Wk_eff^T                   [33, 33]
    M_vo_aug  = [Wv_eff (W_out W_head), e_ones]   [33, 33]
  where Wq_eff = [W_proj W_qkv_q ; b_q_eff] etc. Then:
    scores^T  = Xa^T M_qk Xa      computed as Xa-dot-(M_qk^T Xa), already
                in [s, t] layout, so no PE transposes of the attention
                probabilities are needed.
    VW_aug    = Xa^T M_vo_aug                     [T, 33]
    predu/css = VW_aug^T exp(scores^T/16)         [33, T]
                rows 0..31 = unnormalized pred^T; row 32 = softmax
                denominator (the e_ones column of M_vo_aug sums exp
                columns through the ones row of Xa).
    pred^T    = predu * (1/css) + b_oh
All attention matmuls are bf16 (errors average out in the final scalar
mean); instance-norm, the softmax exp, denominators, and the loss are fp32.
softmax skips the max-subtraction pass: scores/16 ~ N(0,1) and bounded by
|q||k|/16 < ~40, so exp cannot overflow fp32.
sqrt(var) is computed as exp(0.5*ln(var)) so all ScalarE functions come
from one ACT table set (a Sqrt would force ~2.7us table reloads).
"""

import math
import os

import numpy as np

import concourse.bass as bass
import concourse.mybir as mybir
import concourse.tile as tile
from concourse.bass_utils import run_bass_kernel_spmd
from concourse.masks import make_identity, make_upper_triangular
from concourse.vector_clock import ScopedClock

F32 = mybir.dt.float32
BF16 = mybir.dt.bfloat16
AX = mybir.AxisListType
ALU = mybir.AluOpType
AF = mybir.ActivationFunctionType

N_CORES = 8
B = 32
L = 32768
PS = 32
D = 256
T = L // PS  # 1024
BPC = B // N_CORES  # batch elements per core = 4
NT = T // 128  # 8 s-tiles
KA = PS + 1  # augmented contraction dim (extra ones row)
SCALE = 1.0 / math.sqrt(D)  # 1/16


class SplitDrainTileContext(tile.TileContext):
    """TileContext whose final drain splits sem waits across multiple drain
    instructions -- this walrus rejects >1 sync wait per instruction."""

    def _drain_and_barrier(self, tick_clock, wait_clock):
        probe = mybir.InstDrain(name=f"I-{self.nc.next_id()}", ins=[], outs=[])
        probe.engine = mybir.EngineType.SP
        wait_clock.add_sem_waits(probe, ScopedClock({None: tick_clock.global_clock}))
        waits = list(probe.sync_info.on_wait) if probe.sync_info else []
        assert self.sems is not None
        handles = {h.num: h for h in self.sems.allocated().values()}
        if not waits:
            self.nc.sync.drain()
        for w in waits:
            d = self.nc.sync.drain()
            d.wait_op(handles[w.id], w.wait_value, "sem-ge", check=False)
        self.nc.all_engine_barrier()
        popped = self.nc._tile_sem_poison_stack.pop()
        assert popped is self._sem_poison
        self.nc.clear_and_free_semaphores(list(self.sems.allocated().values()))
        self.nc.all_engine_barrier()


def split_excess_waits(nc, max_waits=1):
    """This walrus rejects instructions carrying more than one sync wait.
    Hoist extra waits onto the immediately preceding same-engine
    instruction when that instruction signals nothing (then waiting before
    it cannot starve anyone), else insert a wait-only drain."""
    for f in nc.m.functions:
        for blk in f.blocks:
            insts = list(blk.instructions)
            out = []
            prev_by_engine = {}
            changed = False
            for inst in insts:
                si = inst.sync_info
                waits = list(si.on_wait) if si else []
                if len(waits) > max_waits:
                    changed = True
                    extra, keep = waits[:-max_waits], waits[-max_waits:]
                    remaining = []
                    prev = prev_by_engine.get(str(inst.engine))
                    for w in extra:
                        psi = prev.sync_info if prev is not None else None
                        if prev is not None and (
                            psi is None
                            or (len(psi.on_wait) == 0 and len(psi.on_update) == 0)
                        ):
                            prev.sync_info = mybir.SyncInfo(on_wait=[w], on_update=[])
                            prev = None  # one hoist per predecessor
                        else:
                            remaining.append(w)
                    for w in remaining:
                        dr = mybir.InstDrain(name=f"I-{nc.next_id()}", ins=[], outs=[])
                        dr.engine = inst.engine
                        dr.sync_info = mybir.SyncInfo(on_wait=[w], on_update=[])
                        out.append(dr)
                    inst.sync_info = mybir.SyncInfo(
                        on_wait=keep, on_update=list(si.on_update)
                    )
                out.append(inst)
                prev_by_engine[str(inst.engine)] = inst
            if changed:
                blk.instructions = out


def build_program():
    nc = bass.Bass("TRN2", target_bir_lowering=False, debug=False, num_devices=N_CORES)

    x_d = nc.dram_tensor("x", [BPC, L], F32, kind="ExternalInput")
    mqk_d = nc.dram_tensor("m_qk", [KA, KA], BF16, kind="ExternalInput")
    mvo_d = nc.dram_tensor("m_vo", [KA, KA], BF16, kind="ExternalInput")
    out_d = nc.dram_tensor("loss_partial", [1, 1], F32, kind="ExternalOutput")

    from contextlib import ExitStack

    with SplitDrainTileContext(nc) as tc, ExitStack() as ctx:
        cpool = ctx.enter_context(tc.tile_pool(name="consts", bufs=1))
        ppool_s = ctx.enter_context(tc.tile_pool(name="psum_s", bufs=4, space="PSUM"))
        ppool_o = ctx.enter_context(tc.tile_pool(name="psum_o", bufs=2, space="PSUM"))
        ppool_t = ctx.enter_context(tc.tile_pool(name="psum_t", bufs=2, space="PSUM"))
        xpool = ctx.enter_context(tc.tile_pool(name="xc", bufs=2))
        spool = ctx.enter_context(tc.tile_pool(name="small", bufs=4))
        bigpool = ctx.enter_context(tc.tile_pool(name="big", bufs=2))
        epool = ctx.enter_context(tc.tile_pool(name="et", bufs=10))
        scratch = ctx.enter_context(tc.tile_pool(name="scratch", bufs=3))

        # ---- constants ----
        ident_f = cpool.tile([128, 128], F32)
        make_identity(nc, ident_f[:])
        triu_b = cpool.tile([128, 128], BF16)
        make_upper_triangular(nc, triu_b[:], val=1.0, diag=True)
        ones_col = cpool.tile([128, 1], F32)
        nc.vector.memset(ones_col[:], 1.0)
        ones_row = cpool.tile([1, PS], F32)
        nc.vector.memset(ones_row[:], 1.0)
        ones_row_b = cpool.tile([1, PS], BF16)
        nc.vector.memset(ones_row_b[:], 1.0)

        mqk = cpool.tile([KA, KA], BF16)
        nc.gpsimd.dma_start(mqk[:], mqk_d.ap()[:])
        mvo = cpool.tile([KA, KA], BF16)
        nc.gpsimd.dma_start(mvo[:], mvo_d.ap()[:])

        lp_all = cpool.tile([PS, BPC], F32)  # per-batch loss partials

        for b in range(BPC):
            # ---- A: load x[b] contiguously as [128, 256] ----
            xc = xpool.tile([128, L // 128], F32)
            # partition u, free (k, ps) <- x[b, (128k + u)*32 + ps]: each
            # partition gets 8 tokens at stride 4KB; transposing column block
            # k then yields tokens [128k, 128k+128) contiguously.
            nc.gpsimd.dma_start(
                xc[:].rearrange("u (k ps) -> u k ps", ps=PS),
                x_d.ap()[b].rearrange("(k u ps) -> u k ps", u=128, ps=PS),
            )

            # ---- B: instance-norm stats ----
            sums = spool.tile([128, 2], F32)
            nc.vector.reduce_sum(sums[:, 0:1], xc[:], axis=AX.X)
            sq_scr = scratch.tile([128, L // 128], F32)
            nc.scalar.activation(sq_scr[:], xc[:], AF.Square, accum_out=sums[:, 1:2])
            tot_ps = ppool_s.tile([1, 2], F32, tag="s")
            nc.tensor.matmul(tot_ps[:], ones_col[:], sums[:], start=True, stop=True)
            tot = spool.tile([1, 2], F32)
            nc.vector.tensor_copy(tot[:], tot_ps[:])

            # sc = [mean, s*m, ssq-s*m, ln, std, std+eps, rstd, -m*rstd, -mean]
            sc = spool.tile([1, 9], F32)
            nc.scalar.mul(sc[:, 0:1], tot[:, 0:1], 1.0 / L)  # mean
            nc.vector.tensor_tensor(
                out=sc[:, 1:2], in0=tot[:, 0:1], in1=sc[:, 0:1], op=ALU.mult
            )
            nc.vector.tensor_tensor(
                out=sc[:, 2:3], in0=tot[:, 1:2], in1=sc[:, 1:2], op=ALU.subtract
            )
            nc.scalar.activation(sc[:, 3:4], sc[:, 2:3], AF.Ln, scale=1.0 / (L - 1))
            nc.scalar.activation(sc[:, 4:5], sc[:, 3:4], AF.Exp, scale=0.5)  # std
            nc.vector.tensor_scalar_add(sc[:, 5:6], sc[:, 4:5], 1e-5)
            nc.vector.reciprocal(sc[:, 6:7], sc[:, 5:6])  # rstd
            nc.scalar.mul(sc[:, 8:9], sc[:, 0:1], -1.0)  # -mean
            nc.vector.tensor_tensor(
                out=sc[:, 7:8], in0=sc[:, 8:9], in1=sc[:, 6:7], op=ALU.mult
            )  # -mean*rstd

            # broadcast [rstd, -mean*rstd] to 32 partitions via rank-1 matmul
            bc_ps = ppool_s.tile([PS, 2], F32, tag="s")
            nc.tensor.matmul(bc_ps[:], ones_row[:], sc[:, 6:8], start=True, stop=True)
            bc = spool.tile([PS, 2], F32)
            nc.vector.tensor_copy(bc[:], bc_ps[:])

            # ---- C: transpose x into patch-major Xa [33, 1024], normalized;
            # row 32 is the constant-1 augmentation row ----
            xnt = bigpool.tile([KA, T], F32, tag="xnt")
            nc.gpsimd.memset(xnt[PS : PS + 1, :], 1.0)
            for r in range(2):
                xt_ps = ppool_t.tile([PS, 512], F32, tag="pt")
                for c in range(4):
                    k = 4 * r + c
                    nc.tensor.transpose(
                        xt_ps[:, c * 128 : (c + 1) * 128],
                        xc[:, k * PS : (k + 1) * PS],
                        ident_f[:],
                    )
                # xt_ps[ps, c*128+u] = token 128*(4r+c)+u elem ps: dense write
                nc.vector.tensor_scalar(
                    out=xnt[0:PS, r * 512 : (r + 1) * 512],
                    in0=xt_ps[:],
                    scalar1=bc[:, 0:1],
                    scalar2=bc[:, 1:2],
                    op0=ALU.mult,
                    op1=ALU.add,
                )
            xnt_b = bigpool.tile([KA, T], BF16, tag="xntb")
            nc.any.tensor_copy(xnt_b[:], xnt[:])

            # ---- D: Y = M_qk^T Xa  [33, 1024] bf16 ----
            y_b = bigpool.tile([KA, T], BF16, tag="y")
            for n in range(2):
                y_ps = ppool_s.tile([KA, 512], F32, tag="s")
                nc.tensor.matmul(
                    y_ps[:],
                    mqk[:],
                    xnt_b[:, n * 512 : (n + 1) * 512],
                    start=True,
                    stop=True,
                )
                nc.any.tensor_copy(y_b[:, n * 512 : (n + 1) * 512], y_ps[:])

            # ---- E: VW_aug = Xa^T M_vo_aug  [8][128, 33] bf16 ----
            vw_all = bigpool.tile([128, NT * KA], BF16, tag="vw")
            for j in range(NT):
                vw_ps = ppool_s.tile([128, KA], F32, tag="s")
                nc.tensor.matmul(
                    vw_ps[:],
                    xnt_b[:, j * 128 : (j + 1) * 128],
                    mvo[:],
                    start=True,
                    stop=True,
                )
                nc.any.tensor_copy(vw_all[:, j * KA : (j + 1) * KA], vw_ps[:])

            # ---- F: eT[j] = masked exp(scores^T/16)  [s-tile j][128, j*128..T]
            et = {}
            for j in range(NT):
                e_j = epool.tile([128, T], BF16, tag="et", name=f"et_{b}_{j}")
                et[j] = e_j
                c0 = (j * 128) // 512
                for c in range(c0, 2):
                    off = max(0, j * 128 - c * 512)  # within-chunk start
                    sT_ps = ppool_s.tile([128, 512], F32, tag="s")
                    nc.tensor.matmul(
                        sT_ps[:, off:512],
                        xnt_b[:, j * 128 : (j + 1) * 128],
                        y_b[:, c * 512 + off : (c + 1) * 512],
                        start=True,
                        stop=True,
                    )
                    nc.scalar.activation(
                        e_j[:, c * 512 + off : (c + 1) * 512],
                        sT_ps[:, off:512],
                        AF.Exp,
                        scale=SCALE,
                    )
                    if c == c0:
                        # diagonal block: zero the s > t half (keep s <= t)
                        nc.vector.tensor_tensor(
                            out=e_j[:, j * 128 : (j + 1) * 128],
                            in0=e_j[:, j * 128 : (j + 1) * 128],
                            in1=triu_b[:],
                            op=ALU.mult,
                        )

            # ---- G: [pred_u ; colsum] = VW_aug^T eT, then normalize ----
            predt = bigpool.tile([PS, T], F32, tag="pred")
            for n in range(2):
                pu_ps = ppool_o.tile([KA, 512], F32, tag="o")
                for j in range(4 * n + 4):
                    off = max(0, j * 128 - n * 512)
                    nc.tensor.matmul(
                        pu_ps[:, off:512],
                        vw_all[:, j * KA : (j + 1) * KA],
                        et[j][:, n * 512 + off : (n + 1) * 512],
                        start=(j == 0),
                        stop=(j == 4 * n + 3),
                    )
                # pred = pred_u / colsum (+ b_oh via M_vo bias row);
                # 1/colsum = exp(-ln(colsum)) on ScalarE -- DVE reciprocal is
                # an iterative divide (~4.3 cyc/elem) and was the top DVE cost
                lncs = spool.tile([1, 512], F32, tag="rrow")
                nc.scalar.activation(lncs[:], pu_ps[PS : PS + 1, :], AF.Ln)
                rr_b = spool.tile([1, 512], BF16, tag="rrowb")
                nc.scalar.activation(rr_b[:], lncs[:], AF.Exp, scale=-1.0)
                bcr_ps = ppool_t.tile([PS, 512], F32, tag="pt")
                nc.tensor.matmul(
                    bcr_ps[:], ones_row_b[:], rr_b[:], start=True, stop=True
                )
                bcr_sb = scratch.tile([PS, 512], F32, tag="pn")
                nc.any.tensor_copy(bcr_sb[:], bcr_ps[:])
                nc.vector.tensor_tensor(
                    out=predt[:, n * 512 : (n + 1) * 512],
                    in0=pu_ps[0:PS, :],
                    in1=bcr_sb[:],
                    op=ALU.mult,
                )

            # ---- H: loss partial: sum((pred[:, :-1] - patches[:, 1:])^2) ----
            dd = scratch.tile([PS, T], F32, tag="dd")
            nc.vector.tensor_tensor(
                out=dd[:, 0 : T - 1],
                in0=predt[:, 0 : T - 1],
                in1=xnt[0:PS, 1:T],
                op=ALU.subtract,
            )
            nc.scalar.activation(
                dd[:, 0 : T - 1],
                dd[:, 0 : T - 1],
                AF.Square,
                accum_out=lp_all[:, b : b + 1],
            )

        # ---- final: total partial over batches & partitions ----
        lsum = spool.tile([PS, 1], F32)
        nc.vector.reduce_sum(lsum[:], lp_all[:], axis=AX.X)
        tot_ps2 = ppool_s.tile([1, 1], F32, tag="s")
        nc.tensor.matmul(tot_ps2[:], ones_col[0:PS, :], lsum[:], start=True, stop=True)
        out_sb = spool.tile([1, 1], F32)
        nc.vector.tensor_copy(out_sb[:], tot_ps2[:])
        nc.gpsimd.dma_start(out_d.ap()[:], out_sb[:])

    split_excess_waits(nc)
    return nc


_program_cache = {}


def _get_program():
    if "nc" not in _program_cache:
        _program_cache["nc"] = build_program()
    return _program_cache["nc"]


def make_in_maps(x, W_proj, b_proj, W_qkv, b_qkv, W_out, b_out, W_head, b_head):
    import ml_dtypes

    f8 = np.float64
    w_eff = W_proj.astype(f8) @ W_qkv.astype(f8)  # [32, 768]
    b_eff = b_proj.astype(f8) @ W_qkv.astype(f8) + b_qkv.astype(f8)  # [768]
    w_aug = np.concatenate([w_eff, b_eff[None, :]], axis=0)  # [33, 768]
    wq, wk, wv = w_aug[:, 0:D], w_aug[:, D : 2 * D], w_aug[:, 2 * D : 3 * D]
    m_qk = wq @ wk.T  # [33, 33]
    w_oh = W_out.astype(f8) @ W_head.astype(f8)  # [256, 32]
    b_oh = b_out.astype(f8) @ W_head.astype(f8) + b_head.astype(f8)  # [32]
    m_vo = wv @ w_oh  # [33, 32]
    # folding b_oh into the bias row: pred_u' = sum_s (VW + b_oh) eT, so
    # pred_u'/colsum = pred + b_oh exactly.
    m_vo[PS, :] += b_oh
    e_ones = np.zeros((KA, 1), f8)
    e_ones[PS, 0] = 1.0  # selects Xa's ones row -> colsum output column
    m_vo_aug = np.concatenate([m_vo, e_ones], axis=1)  # [33, 33]

    mqk_b = np.ascontiguousarray(m_qk.astype(ml_dtypes.bfloat16))
    mvo_b = np.ascontiguousarray(m_vo_aug.astype(ml_dtypes.bfloat16))

    in_maps = []
    for core in range(N_CORES):
        xs = np.ascontiguousarray(x[core * BPC : (core + 1) * BPC])
        in_maps.append({"x": xs, "m_qk": mqk_b, "m_vo": mvo_b})
    return in_maps


def kernel(**inputs) -> np.ndarray:
    inputs = {k: np.asarray(v) for k, v in inputs.items()}
    nc = _get_program()
    in_maps = make_in_maps(**inputs)
    res = run_bass_kernel_spmd(nc, in_maps, core_ids=list(range(N_CORES)))
    total = sum(float(res.results[i]["loss_partial"][0, 0]) for i in range(N_CORES))
    loss = total / (B * (T - 1) * PS)
    return np.float32(loss)


if __name__ == "__main__":
    rng = np.random.default_rng(0)
    ins = {
        "x": rng.standard_normal((B, L)).astype(np.float32),
        "W_proj": (rng.standard_normal((PS, D)) / math.sqrt(PS)).astype(np.float32),
        "b_proj": np.zeros(D, np.float32),
        "W_qkv": (rng.standard_normal((D, 3 * D)) / math.sqrt(D)).astype(np.float32),
        "b_qkv": np.zeros(3 * D, np.float32),
        "W_out": (rng.standard_normal((D, D)) / math.sqrt(D)).astype(np.float32),
        "b_out": np.zeros(D, np.float32),
        "W_head": (rng.standard_normal((D, PS)) / math.sqrt(D)).astype(np.float32),
        "b_head": np.zeros(PS, np.float32),
    }
    got = kernel(**ins)
    print("kernel loss:", got)


# revision 27
# speedup vs baseline: 1.8882x; 1.0161x over previous
"""Trainium2 Bass kernel for nn_AttentionModel (patch-transformer + MSE loss).

Model (per batch element b of B=32):
    x[b] : [L=32768] --instance-norm--> patches [T=1024, PS=32]
    h    = patches @ W_proj + b_proj                  [T, 256]
    qkv  = h @ W_qkv + b_qkv ;  q,k,v = split(qkv)    [T, 256] each
    attn = softmax(causal(q k^T / 16))                [T, T]
    out  = (attn @ v) @ W_out + b_out                 [T, 256]
    pred = out @ W_head + b_head                      [T, PS]
    loss = mean((pred[:, :-1] - patches[:, 1:])**2)   scalar

Sharding: data-parallel over batch, 4 batch elements per core x 8 cores.
Each core computes a partial sum-of-squares; host combines.

Key algebraic restructure (host-precomputed constants):
    Xa        = [patches^T ; ones]                [33, T] (normalized on-chip)
    M_qk      = Wq_eff Wk_eff^T                   [33, 33]
    M_vo_aug  = [Wv_eff (W_out W_head), e_ones]   [33, 33]
  where Wq_eff = [W_proj W_qkv_q ; b_q_eff] etc. Then:
    scores^T  = Xa^T M_qk Xa      computed as Xa-dot-(M_qk^T Xa), already
                in [s, t] layout, so no PE transposes of the attention
                probabilities are needed.
    VW_aug    = Xa^T M_vo_aug                     [T, 33]
    predu/css = VW_aug^T exp(scores^T/16)         [33, T]
                rows 0..31 = unnormalized pred^T; row 32 = softmax
                denominator (the e_ones column of M_vo_aug sums exp
                columns through the ones row of Xa).
    pred^T    = predu * (1/css) + b_oh
All attention matmuls are bf16 (errors average out in the final scalar
mean); instance-norm, the softmax exp, denominators, and the loss are fp32.
softmax skips the max-subtraction pass: scores/16 ~ N(0,1) and bounded by
|q||k|/16 < ~40, so exp cannot overflow fp32.
sqrt(var) is computed as exp(0.5*ln(var)) so all ScalarE functions come
from one ACT table set (a Sqrt would force ~2.7us table reloads).
"""

import math
import os

import numpy as np

import concourse.bass as bass
import concourse.mybir as mybir
import concourse.tile as tile
from concourse.bass_utils import run_bass_kernel_spmd
from concourse.masks import make_identity, make_upper_triangular
from concourse.vector_clock import ScopedClock

F32 = mybir.dt.float32
BF16 = mybir.dt.bfloat16
AX = mybir.AxisListType
ALU = mybir.AluOpType
AF = mybir.ActivationFunctionType

N_CORES = 8
B = 32
L = 32768
PS = 32
D = 256
T = L // PS  # 1024
BPC = B // N_CORES  # batch elements per core = 4
NT = T // 128  # 8 s-tiles
KA = PS + 1  # augmented contraction dim (extra ones row)
SCALE = 1.0 / math.sqrt(D)  # 1/16


class SplitDrainTileContext(tile.TileContext):
    """TileContext whose final drain splits sem waits across multiple drain
    instructions -- this walrus rejects >1 sync wait per instruction."""

    def _drain_and_barrier(self, tick_clock, wait_clock):
        probe = mybir.InstDrain(name=f"I-{self.nc.next_id()}", ins=[], outs=[])
        probe.engine = mybir.EngineType.SP
        wait_clock.add_sem_waits(probe, ScopedClock({None: tick_clock.global_clock}))
        waits = list(probe.sync_info.on_wait) if probe.sync_info else []
        assert self.sems is not None
        handles = {h.num: h for h in self.sems.allocated().values()}
        if not waits:
            self.nc.sync.drain()
        for w in waits:
            d = self.nc.sync.drain()
            d.wait_op(handles[w.id], w.wait_value, "sem-ge", check=False)
        self.nc.all_engine_barrier()
        popped = self.nc._tile_sem_poison_stack.pop()
        assert popped is self._sem_poison
        self.nc.clear_and_free_semaphores(list(self.sems.allocated().values()))
        self.nc.all_engine_barrier()


def split_excess_waits(nc, max_waits=1):
    """This walrus rejects instructions carrying more than one sync wait.
    Hoist extra waits onto the immediately preceding same-engine
    instruction when that instruction signals nothing (then waiting before
    it cannot starve anyone), else insert a wait-only drain."""
    for f in nc.m.functions:
        for blk in f.blocks:
            insts = list(blk.instructions)
            out = []
            prev_by_engine = {}
            changed = False
            for inst in insts:
                si = inst.sync_info
                waits = list(si.on_wait) if si else []
                if len(waits) > max_waits:
                    changed = True
                    extra, keep = waits[:-max_waits], waits[-max_waits:]
                    remaining = []
                    prev = prev_by_engine.get(str(inst.engine))
                    for w in extra:
                        psi = prev.sync_info if prev is not None else None
                        if prev is not None and (
                            psi is None
                            or (len(psi.on_wait) == 0 and len(psi.on_update) == 0)
                        ):
                            prev.sync_info = mybir.SyncInfo(on_wait=[w], on_update=[])
                            prev = None  # one hoist per predecessor
                        else:
                            remaining.append(w)
                    for w in remaining:
                        dr = mybir.InstDrain(name=f"I-{nc.next_id()}", ins=[], outs=[])
                        dr.engine = inst.engine
                        dr.sync_info = mybir.SyncInfo(on_wait=[w], on_update=[])
                        out.append(dr)
                    inst.sync_info = mybir.SyncInfo(
                        on_wait=keep, on_update=list(si.on_update)
                    )
                out.append(inst)
                prev_by_engine[str(inst.engine)] = inst
            if changed:
                blk.instructions = out


def build_program():
    nc = bass.Bass("TRN2", target_bir_lowering=False, debug=False, num_devices=N_CORES)

    x_d = nc.dram_tensor("x", [BPC, L], F32, kind="ExternalInput")
    mqk_d = nc.dram_tensor("m_qk", [KA, KA], BF16, kind="ExternalInput")
    mvo_d = nc.dram_tensor("m_vo", [KA, KA], BF16, kind="ExternalInput")
    out_d = nc.dram_tensor("loss_partial", [1, 1], F32, kind="ExternalOutput")

    from contextlib import ExitStack

    with SplitDrainTileContext(nc) as tc, ExitStack() as ctx:
        cpool = ctx.enter_context(tc.tile_pool(name="consts", bufs=1))
        ppool_s = ctx.enter_context(tc.tile_pool(name="psum_s", bufs=4, space="PSUM"))
        ppool_o = ctx.enter_context(tc.tile_pool(name="psum_o", bufs=2, space="PSUM"))
        ppool_t = ctx.enter_context(tc.tile_pool(name="psum_t", bufs=2, space="PSUM"))
        xpool = ctx.enter_context(tc.tile_pool(name="xc", bufs=2))
        spool = ctx.enter_context(tc.tile_pool(name="small", bufs=4))
        bigpool = ctx.enter_context(tc.tile_pool(name="big", bufs=2))
        epool = ctx.enter_context(tc.tile_pool(name="et", bufs=10))
        scratch = ctx.enter_context(tc.tile_pool(name="scratch", bufs=3))

        # ---- constants ----
        ident_f = cpool.tile([128, 128], F32)
        make_identity(nc, ident_f[:])
        triu_b = cpool.tile([128, 128], BF16)
        make_upper_triangular(nc, triu_b[:], val=1.0, diag=True)
        ones_col = cpool.tile([128, 1], F32)
        nc.vector.memset(ones_col[:], 1.0)
        ones_row = cpool.tile([1, PS], F32)
        nc.vector.memset(ones_row[:], 1.0)
        ones_row_b = cpool.tile([1, PS], BF16)
        nc.vector.memset(ones_row_b[:], 1.0)

        mqk = cpool.tile([KA, KA], BF16)
        nc.gpsimd.dma_start(mqk[:], mqk_d.ap()[:])
        mvo = cpool.tile([KA, KA], BF16)
        nc.gpsimd.dma_start(mvo[:], mvo_d.ap()[:])

        lp_all = cpool.tile([PS, BPC], F32)  # per-batch loss partials

        for b in range(BPC):
            # ---- A: load x[b] contiguously as [128, 256] ----
            xc = xpool.tile([128, L // 128], F32)
            # partition u, free (k, ps) <- x[b, (128k + u)*32 + ps]: each
            # partition gets 8 tokens at stride 4KB; transposing column block
            # k then yields tokens [128k, 128k+128) contiguously.
            nc.gpsimd.dma_start(
                xc[:].rearrange("u (k ps) -> u k ps", ps=PS),
                x_d.ap()[b].rearrange("(k u ps) -> u k ps", u=128, ps=PS),
            )

            # ---- B: instance-norm stats ----
            sums = spool.tile([128, 2], F32)
            nc.vector.reduce_sum(sums[:, 0:1], xc[:], axis=AX.X)
            sq_scr = scratch.tile([128, L // 128], F32)
            nc.vector.tensor_tensor(out=sq_scr[:], in0=xc[:], in1=xc[:], op=ALU.mult)
            nc.vector.reduce_sum(sums[:, 1:2], sq_scr[:], axis=AX.X)
            tot_ps = ppool_s.tile([1, 2], F32, tag="s")
            nc.tensor.matmul(tot_ps[:], ones_col[:], sums[:], start=True, stop=True)
            tot = spool.tile([1, 2], F32)
            nc.vector.tensor_copy(tot[:], tot_ps[:])

            # sc = [mean, s*m, ssq-s*m, ln, std, std+eps, rstd, -m*rstd, -mean]
            sc = spool.tile([1, 9], F32)
            nc.scalar.mul(sc[:, 0:1], tot[:, 0:1], 1.0 / L)  # mean
            nc.vector.tensor_tensor(
                out=sc[:, 1:2], in0=tot[:, 0:1], in1=sc[:, 0:1], op=ALU.mult
            )
            nc.vector.tensor_tensor(
                out=sc[:, 2:3], in0=tot[:, 1:2], in1=sc[:, 1:2], op=ALU.subtract
            )
            nc.scalar.activation(sc[:, 3:4], sc[:, 2:3], AF.Ln, scale=1.0 / (L - 1))
            nc.scalar.activation(sc[:, 4:5], sc[:, 3:4], AF.Exp, scale=0.5)  # std
            nc.vector.tensor_scalar_add(sc[:, 5:6], sc[:, 4:5], 1e-5)
            nc.vector.reciprocal(sc[:, 6:7], sc[:, 5:6])  # rstd
            nc.scalar.mul(sc[:, 8:9], sc[:, 0:1], -1.0)  # -mean
            nc.vector.tensor_tensor(
                out=sc[:, 7:8], in0=sc[:, 8:9], in1=sc[:, 6:7], op=ALU.mult
            )  # -mean*rstd

            # broadcast [rstd, -mean*rstd] to 32 partitions via rank-1 matmul
            bc_ps = ppool_s.tile([PS, 2], F32, tag="s")
            nc.tensor.matmul(bc_ps[:], ones_row[:], sc[:, 6:8], start=True, stop=True)
            bc = spool.tile([PS, 2], F32)
            nc.vector.tensor_copy(bc[:], bc_ps[:])

            # ---- C: transpose x into patch-major Xa [33, 1024], normalized;
            # row 32 is the constant-1 augmentation row ----
            xnt_b = bigpool.tile([KA, T], BF16, tag="xntb")
            nc.gpsimd.memset(xnt_b[PS : PS + 1, :], 1.0)
            for r in range(2):
                xt_ps = ppool_t.tile([PS, 512], F32, tag="pt")
                for c in range(4):
                    k = 4 * r + c
                    nc.tensor.transpose(
                        xt_ps[:, c * 128 : (c + 1) * 128],
                        xc[:, k * PS : (k + 1) * PS],
                        ident_f[:],
                    )
                # xt_ps[ps, c*128+u] = token 128*(4r+c)+u elem ps: dense write
                nc.vector.tensor_scalar(
                    out=xnt_b[0:PS, r * 512 : (r + 1) * 512],
                    in0=xt_ps[:],
                    scalar1=bc[:, 0:1],
                    scalar2=bc[:, 1:2],
                    op0=ALU.mult,
                    op1=ALU.add,
                )
            # ---- D: Y = M_qk^T Xa  [33, 1024] bf16 ----
            y_b = bigpool.tile([KA, T], BF16, tag="y")
            for n in range(2):
                y_ps = ppool_s.tile([KA, 512], F32, tag="s")
                nc.tensor.matmul(
                    y_ps[:],
                    mqk[:],
                    xnt_b[:, n * 512 : (n + 1) * 512],
                    start=True,
                    stop=True,
                )
                nc.vector.tensor_copy(y_b[:, n * 512 : (n + 1) * 512], y_ps[:])

            # ---- E: VW_aug = Xa^T M_vo_aug  [8][128, 33] bf16 ----
            vw_all = bigpool.tile([128, NT * KA], BF16, tag="vw")
            for h in range(2):
                vw_ps = ppool_s.tile([128, 4 * KA], F32, tag="s")
                for jj in range(4):
                    j = 4 * h + jj
                    nc.tensor.matmul(
                        vw_ps[:, jj * KA : (jj + 1) * KA],
                        xnt_b[:, j * 128 : (j + 1) * 128],
                        mvo[:],
                        start=True,
                        stop=True,
                    )
                nc.vector.tensor_copy(
                    vw_all[:, h * 4 * KA : (h + 1) * 4 * KA], vw_ps[:]
                )

            # ---- F: eT[j] = masked exp(scores^T/16)  [s-tile j][128, j*128..T]
            et = {}
            for j in range(NT):
                e_j = epool.tile([128, T], BF16, tag="et", name=f"et_{b}_{j}")
                et[j] = e_j
                c0 = (j * 128) // 512
                for c in range(c0, 2):
                    off = max(0, j * 128 - c * 512)  # within-chunk start
                    sT_ps = ppool_s.tile([128, 512], F32, tag="s")
                    nc.tensor.matmul(
                        sT_ps[:, off:512],
                        xnt_b[:, j * 128 : (j + 1) * 128],
                        y_b[:, c * 512 + off : (c + 1) * 512],
                        start=True,
                        stop=True,
                    )
                    nc.scalar.activation(
                        e_j[:, c * 512 + off : (c + 1) * 512],
                        sT_ps[:, off:512],
                        AF.Exp,
                        scale=SCALE,
                    )
                    if c == c0:
                        # diagonal block: zero the s > t half (keep s <= t)
                        nc.vector.tensor_tensor(
                            out=e_j[:, j * 128 : (j + 1) * 128],
                            in0=e_j[:, j * 128 : (j + 1) * 128],
                            in1=triu_b[:],
                            op=ALU.mult,
                        )

            # ---- G: [pred_u ; colsum] = VW_aug^T eT, then normalize ----
            predt = bigpool.tile([PS, T], F32, tag="pred")
            for n in range(2):
                pu_ps = ppool_o.tile([KA, 512], F32, tag="o")
                for j in range(4 * n + 4):
                    off = max(0, j * 128 - n * 512)
                    nc.tensor.matmul(
                        pu_ps[:, off:512],
                        vw_all[:, j * KA : (j + 1) * KA],
                        et[j][:, n * 512 + off : (n + 1) * 512],
                        start=(j == 0),
                        stop=(j == 4 * n + 3),
                    )
                # pred = pred_u / colsum (+ b_oh via M_vo bias row);
                # 1/colsum = exp(-ln(colsum)) on ScalarE -- DVE reciprocal is
                # an iterative divide (~4.3 cyc/elem) and was the top DVE cost
                lncs = spool.tile([1, 512], F32, tag="rrow")
                nc.scalar.activation(lncs[:], pu_ps[PS : PS + 1, :], AF.Ln)
                rr_b = spool.tile([1, 512], BF16, tag="rrowb")
                nc.scalar.activation(rr_b[:], lncs[:], AF.Exp, scale=-1.0)
                bcr_ps = ppool_t.tile([PS, 512], F32, tag="pt")
                nc.tensor.matmul(
                    bcr_ps[:], ones_row_b[:], rr_b[:], start=True, stop=True
                )
                bcr_sb = scratch.tile([PS, 512], F32, tag="pn")
                nc.vector.tensor_copy(bcr_sb[:], bcr_ps[:])
                nc.vector.tensor_tensor(
                    out=predt[:, n * 512 : (n + 1) * 512],
                    in0=pu_ps[0:PS, :],
                    in1=bcr_sb[:],
                    op=ALU.mult,
                )

            # ---- H: loss partial: sum((pred[:, :-1] - patches[:, 1:])^2) ----
            dd = scratch.tile([PS, T], F32, tag="dd")
            nc.vector.tensor_tensor(
                out=dd[:, 0 : T - 1],
                in0=predt[:, 0 : T - 1],
                in1=xnt_b[0:PS, 1:T],
                op=ALU.subtract,
            )
            nc.scalar.activation(
                dd[:, 0 : T - 1],
                dd[:, 0 : T - 1],
                AF.Square,
                accum_out=lp_all[:, b : b + 1],
            )

        # ---- final: total partial over batches & partitions ----
        lsum = spool.tile([PS, 1], F32)
        nc.vector.reduce_sum(lsum[:], lp_all[:], axis=AX.X)
        tot_ps2 = ppool_s.tile([1, 1], F32, tag="s")
        nc.tensor.matmul(tot_ps2[:], ones_col[0:PS, :], lsum[:], start=True, stop=True)
        out_sb = spool.tile([1, 1], F32)
        nc.vector.tensor_copy(out_sb[:], tot_ps2[:])
        nc.gpsimd.dma_start(out_d.ap()[:], out_sb[:])

    split_excess_waits(nc)
    return nc


_program_cache = {}


def _get_program():
    if "nc" not in _program_cache:
        _program_cache["nc"] = build_program()
    return _program_cache["nc"]


def make_in_maps(x, W_proj, b_proj, W_qkv, b_qkv, W_out, b_out, W_head, b_head):
    import ml_dtypes

    f8 = np.float64
    w_eff = W_proj.astype(f8) @ W_qkv.astype(f8)  # [32, 768]
    b_eff = b_proj.astype(f8) @ W_qkv.astype(f8) + b_qkv.astype(f8)  # [768]
    w_aug = np.concatenate([w_eff, b_eff[None, :]], axis=0)  # [33, 768]
    wq, wk, wv = w_aug[:, 0:D], w_aug[:, D : 2 * D], w_aug[:, 2 * D : 3 * D]
    m_qk = wq @ wk.T  # [33, 33]
    w_oh = W_out.astype(f8) @ W_head.astype(f8)  # [256, 32]
    b_oh = b_out.astype(f8) @ W_head.astype(f8) + b_head.astype(f8)  # [32]
    m_vo = wv @ w_oh  # [33, 32]
    # folding b_oh into the bias row: pred_u' = sum_s (VW + b_oh) eT, so
    # pred_u'/colsum = pred + b_oh exactly.
    m_vo[PS, :] += b_oh
    e_ones = np.zeros((KA, 1), f8)
    e_ones[PS, 0] = 1.0  # selects Xa's ones row -> colsum output column
    m_vo_aug = np.concatenate([m_vo, e_ones], axis=1)  # [33, 33]

    mqk_b = np.ascontiguousarray(m_qk.astype(ml_dtypes.bfloat16))
    mvo_b = np.ascontiguousarray(m_vo_aug.astype(ml_dtypes.bfloat16))

    in_maps = []
    for core in range(N_CORES):
        xs = np.ascontiguousarray(x[core * BPC : (core + 1) * BPC])
        in_maps.append({"x": xs, "m_qk": mqk_b, "m_vo": mvo_b})
    return in_maps


def kernel(**inputs) -> np.ndarray:
    inputs = {k: np.asarray(v) for k, v in inputs.items()}
    nc = _get_program()
    in_maps = make_in_maps(**inputs)
    res = run_bass_kernel_spmd(nc, in_maps, core_ids=list(range(N_CORES)))
    total = sum(float(res.results[i]["loss_partial"][0, 0]) for i in range(N_CORES))
    loss = total / (B * (T - 1) * PS)
    return np.float32(loss)


if __name__ == "__main__":
    rng = np.random.default_rng(0)
    ins = {
        "x": rng.standard_normal((B, L)).astype(np.float32),
        "W_proj": (rng.standard_normal((PS, D)) / math.sqrt(PS)).astype(np.float32),
        "b_proj": np.zeros(D, np.float32),
        "W_qkv": (rng.standard_normal((D, 3 * D)) / math.sqrt(D)).astype(np.float32),
        "b_qkv": np.zeros(3 * D, np.float32),
        "W_out": (rng.standard_normal((D, D)) / math.sqrt(D)).astype(np.float32),
        "b_out": np.zeros(D, np.float32),
        "W_head": (rng.standard_normal((D, PS)) / math.sqrt(D)).astype(np.float32),
        "b_head": np.zeros(PS, np.float32),
    }
    got = kernel(**ins)
    print("kernel loss:", got)


# revision 28
# speedup vs baseline: 1.9132x; 1.0133x over previous
"""Trainium2 Bass kernel for nn_AttentionModel (patch-transformer + MSE loss).

Model (per batch element b of B=32):
    x[b] : [L=32768] --instance-norm--> patches [T=1024, PS=32]
    h    = patches @ W_proj + b_proj                  [T, 256]
    qkv  = h @ W_qkv + b_qkv ;  q,k,v = split(qkv)    [T, 256] each
    attn = softmax(causal(q k^T / 16))                [T, T]
    out  = (attn @ v) @ W_out + b_out                 [T, 256]
    pred = out @ W_head + b_head                      [T, PS]
    loss = mean((pred[:, :-1] - patches[:, 1:])**2)   scalar

Sharding: data-parallel over batch, 4 batch elements per core x 8 cores.
Each core computes a partial sum-of-squares; host combines.

Key algebraic restructure (host-precomputed constants):
    Xa        = [patches^T ; ones]                [33, T] (normalized on-chip)
    M_qk      = Wq_eff Wk_eff^T                   [33, 33]
    M_vo_aug  = [Wv_eff (W_out W_head), e_ones]   [33, 33]
  where Wq_eff = [W_proj W_qkv_q ; b_q_eff] etc. Then:
    scores^T  = Xa^T M_qk Xa      computed as Xa-dot-(M_qk^T Xa), already
                in [s, t] layout, so no PE transposes of the attention
                probabilities are needed.
    VW_aug    = Xa^T M_vo_aug                     [T, 33]
    predu/css = VW_aug^T exp(scores^T/16)         [33, T]
                rows 0..31 = unnormalized pred^T; row 32 = softmax
                denominator (the e_ones column of M_vo_aug sums exp
                columns through the ones row of Xa).
    pred^T    = predu * (1/css) + b_oh
All attention matmuls are bf16 (errors average out in the final scalar
mean); instance-norm, the softmax exp, denominators, and the loss are fp32.
softmax skips the max-subtraction pass: scores/16 ~ N(0,1) and bounded by
|q||k|/16 < ~40, so exp cannot overflow fp32.
sqrt(var) is computed as exp(0.5*ln(var)) so all ScalarE functions come
from one ACT table set (a Sqrt would force ~2.7us table reloads).
"""

import math
import os

import numpy as np

import concourse.bass as bass
import concourse.mybir as mybir
import concourse.tile as tile
from concourse.bass_utils import run_bass_kernel_spmd
from concourse.masks import make_identity, make_upper_triangular
from concourse.vector_clock import ScopedClock

F32 = mybir.dt.float32
BF16 = mybir.dt.bfloat16
AX = mybir.AxisListType
ALU = mybir.AluOpType
AF = mybir.ActivationFunctionType

N_CORES = 8
B = 32
L = 32768
PS = 32
D = 256
T = L // PS  # 1024
BPC = B // N_CORES  # batch elements per core = 4
NT = T // 128  # 8 s-tiles
KA = PS + 1  # augmented contraction dim (extra ones row)
SCALE = 1.0 / math.sqrt(D)  # 1/16


class SplitDrainTileContext(tile.TileContext):
    """TileContext whose final drain splits sem waits across multiple drain
    instructions -- this walrus rejects >1 sync wait per instruction."""

    def _drain_and_barrier(self, tick_clock, wait_clock):
        probe = mybir.InstDrain(name=f"I-{self.nc.next_id()}", ins=[], outs=[])
        probe.engine = mybir.EngineType.SP
        wait_clock.add_sem_waits(probe, ScopedClock({None: tick_clock.global_clock}))
        waits = list(probe.sync_info.on_wait) if probe.sync_info else []
        assert self.sems is not None
        handles = {h.num: h for h in self.sems.allocated().values()}
        if not waits:
            self.nc.sync.drain()
        for w in waits:
            d = self.nc.sync.drain()
            d.wait_op(handles[w.id], w.wait_value, "sem-ge", check=False)
        self.nc.all_engine_barrier()
        popped = self.nc._tile_sem_poison_stack.pop()
        assert popped is self._sem_poison
        self.nc.clear_and_free_semaphores(list(self.sems.allocated().values()))
        self.nc.all_engine_barrier()


def split_excess_waits(nc, max_waits=1):
    """This walrus rejects instructions carrying more than one sync wait.
    Hoist extra waits onto the immediately preceding same-engine
    instruction when that instruction signals nothing (then waiting before
    it cannot starve anyone), else insert a wait-only drain."""
    for f in nc.m.functions:
        for blk in f.blocks:
            insts = list(blk.instructions)
            out = []
            prev_by_engine = {}
            changed = False
            for inst in insts:
                si = inst.sync_info
                waits = list(si.on_wait) if si else []
                if len(waits) > max_waits:
                    changed = True
                    extra, keep = waits[:-max_waits], waits[-max_waits:]
                    remaining = []
                    prev = prev_by_engine.get(str(inst.engine))
                    for w in extra:
                        psi = prev.sync_info if prev is not None else None
                        if prev is not None and (
                            psi is None
                            or (len(psi.on_wait) == 0 and len(psi.on_update) == 0)
                        ):
                            prev.sync_info = mybir.SyncInfo(on_wait=[w], on_update=[])
                            prev = None  # one hoist per predecessor
                        else:
                            remaining.append(w)
                    for w in remaining:
                        dr = mybir.InstDrain(name=f"I-{nc.next_id()}", ins=[], outs=[])
                        dr.engine = inst.engine
                        dr.sync_info = mybir.SyncInfo(on_wait=[w], on_update=[])
                        out.append(dr)
                    inst.sync_info = mybir.SyncInfo(
                        on_wait=keep, on_update=list(si.on_update)
                    )
                out.append(inst)
                prev_by_engine[str(inst.engine)] = inst
            if changed:
                blk.instructions = out


def build_program():
    nc = bass.Bass("TRN2", target_bir_lowering=False, debug=False, num_devices=N_CORES)

    x_d = nc.dram_tensor("x", [BPC, L], F32, kind="ExternalInput")
    mqk_d = nc.dram_tensor("m_qk", [KA, KA], BF16, kind="ExternalInput")
    mvo_d = nc.dram_tensor("m_vo", [KA, KA], BF16, kind="ExternalInput")
    out_d = nc.dram_tensor("loss_partial", [1, 1], F32, kind="ExternalOutput")

    from contextlib import ExitStack

    with SplitDrainTileContext(nc) as tc, ExitStack() as ctx:
        cpool = ctx.enter_context(tc.tile_pool(name="consts", bufs=1))
        ppool_s = ctx.enter_context(tc.tile_pool(name="psum_s", bufs=4, space="PSUM"))
        ppool_o = ctx.enter_context(tc.tile_pool(name="psum_o", bufs=2, space="PSUM"))
        ppool_t = ctx.enter_context(tc.tile_pool(name="psum_t", bufs=2, space="PSUM"))
        xpool = ctx.enter_context(tc.tile_pool(name="xc", bufs=3))
        spool = ctx.enter_context(tc.tile_pool(name="small", bufs=6))
        bigpool = ctx.enter_context(tc.tile_pool(name="big", bufs=3))
        epool = ctx.enter_context(tc.tile_pool(name="et", bufs=18))
        scratch = ctx.enter_context(tc.tile_pool(name="scratch", bufs=4))

        # ---- constants ----
        ident_f = cpool.tile([128, 128], F32)
        make_identity(nc, ident_f[:])
        triu_b = cpool.tile([128, 128], BF16)
        make_upper_triangular(nc, triu_b[:], val=1.0, diag=True)
        ones_col = cpool.tile([128, 1], F32)
        nc.vector.memset(ones_col[:], 1.0)
        ones_row = cpool.tile([1, PS], F32)
        nc.vector.memset(ones_row[:], 1.0)
        ones_row_b = cpool.tile([1, PS], BF16)
        nc.vector.memset(ones_row_b[:], 1.0)

        mqk = cpool.tile([KA, KA], BF16)
        nc.gpsimd.dma_start(mqk[:], mqk_d.ap()[:])
        mvo = cpool.tile([KA, KA], BF16)
        nc.gpsimd.dma_start(mvo[:], mvo_d.ap()[:])

        lp_all = cpool.tile([PS, BPC], F32)  # per-batch loss partials

        for b in range(BPC):
            # ---- A: load x[b] contiguously as [128, 256] ----
            xc = xpool.tile([128, L // 128], F32)
            # partition u, free (k, ps) <- x[b, (128k + u)*32 + ps]: each
            # partition gets 8 tokens at stride 4KB; transposing column block
            # k then yields tokens [128k, 128k+128) contiguously.
            nc.gpsimd.dma_start(
                xc[:].rearrange("u (k ps) -> u k ps", ps=PS),
                x_d.ap()[b].rearrange("(k u ps) -> u k ps", u=128, ps=PS),
            )

            # ---- B: instance-norm stats ----
            sums = spool.tile([128, 2], F32)
            nc.vector.reduce_sum(sums[:, 0:1], xc[:], axis=AX.X)
            sq_scr = scratch.tile([128, L // 128], F32)
            nc.vector.tensor_tensor(out=sq_scr[:], in0=xc[:], in1=xc[:], op=ALU.mult)
            nc.vector.reduce_sum(sums[:, 1:2], sq_scr[:], axis=AX.X)
            tot_ps = ppool_s.tile([1, 2], F32, tag="s")
            nc.tensor.matmul(tot_ps[:], ones_col[:], sums[:], start=True, stop=True)
            tot = spool.tile([1, 2], F32)
            nc.vector.tensor_copy(tot[:], tot_ps[:])

            # sc = [mean, s*m, ssq-s*m, ln, std, std+eps, rstd, -m*rstd, -mean]
            sc = spool.tile([1, 9], F32)
            nc.scalar.mul(sc[:, 0:1], tot[:, 0:1], 1.0 / L)  # mean
            nc.vector.tensor_tensor(
                out=sc[:, 1:2], in0=tot[:, 0:1], in1=sc[:, 0:1], op=ALU.mult
            )
            nc.vector.tensor_tensor(
                out=sc[:, 2:3], in0=tot[:, 1:2], in1=sc[:, 1:2], op=ALU.subtract
            )
            nc.scalar.activation(sc[:, 3:4], sc[:, 2:3], AF.Ln, scale=1.0 / (L - 1))
            nc.scalar.activation(sc[:, 4:5], sc[:, 3:4], AF.Exp, scale=0.5)  # std
            nc.vector.tensor_scalar_add(sc[:, 5:6], sc[:, 4:5], 1e-5)
            nc.vector.reciprocal(sc[:, 6:7], sc[:, 5:6])  # rstd
            nc.scalar.mul(sc[:, 8:9], sc[:, 0:1], -1.0)  # -mean
            nc.vector.tensor_tensor(
                out=sc[:, 7:8], in0=sc[:, 8:9], in1=sc[:, 6:7], op=ALU.mult
            )  # -mean*rstd

            # broadcast [rstd, -mean*rstd] to 32 partitions via rank-1 matmul
            bc_ps = ppool_s.tile([PS, 2], F32, tag="s")
            nc.tensor.matmul(bc_ps[:], ones_row[:], sc[:, 6:8], start=True, stop=True)
            bc = spool.tile([PS, 2], F32)
            nc.vector.tensor_copy(bc[:], bc_ps[:])

            # ---- C: transpose x into patch-major Xa [33, 1024], normalized;
            # row 32 is the constant-1 augmentation row ----
            xnt_b = bigpool.tile([KA, T], BF16, tag="xntb")
            nc.gpsimd.memset(xnt_b[PS : PS + 1, :], 1.0)
            for r in range(2):
                xt_ps = ppool_t.tile([PS, 512], F32, tag="pt")
                for c in range(4):
                    k = 4 * r + c
                    nc.tensor.transpose(
                        xt_ps[:, c * 128 : (c + 1) * 128],
                        xc[:, k * PS : (k + 1) * PS],
                        ident_f[:],
                    )
                # xt_ps[ps, c*128+u] = token 128*(4r+c)+u elem ps: dense write
                nc.vector.tensor_scalar(
                    out=xnt_b[0:PS, r * 512 : (r + 1) * 512],
                    in0=xt_ps[:],
                    scalar1=bc[:, 0:1],
                    scalar2=bc[:, 1:2],
                    op0=ALU.mult,
                    op1=ALU.add,
                )
            # ---- D: Y = M_qk^T Xa  [33, 1024] bf16 ----
            y_b = bigpool.tile([KA, T], BF16, tag="y")
            for n in range(2):
                y_ps = ppool_s.tile([KA, 512], F32, tag="s")
                nc.tensor.matmul(
                    y_ps[:],
                    mqk[:],
                    xnt_b[:, n * 512 : (n + 1) * 512],
                    start=True,
                    stop=True,
                )
                nc.vector.tensor_copy(y_b[:, n * 512 : (n + 1) * 512], y_ps[:])

            # ---- E: VW_aug = Xa^T M_vo_aug  [8][128, 33] bf16 ----
            vw_all = bigpool.tile([128, NT * KA], BF16, tag="vw")
            for h in range(2):
                vw_ps = ppool_s.tile([128, 4 * KA], F32, tag="s")
                for jj in range(4):
                    j = 4 * h + jj
                    nc.tensor.matmul(
                        vw_ps[:, jj * KA : (jj + 1) * KA],
                        xnt_b[:, j * 128 : (j + 1) * 128],
                        mvo[:],
                        start=True,
                        stop=True,
                    )
                nc.vector.tensor_copy(
                    vw_all[:, h * 4 * KA : (h + 1) * 4 * KA], vw_ps[:]
                )

            # ---- F: eT[j] = masked exp(scores^T/16)  [s-tile j][128, j*128..T]
            et = {}
            for j in range(NT):
                e_j = epool.tile([128, T], BF16, tag="et", name=f"et_{b}_{j}")
                et[j] = e_j
                c0 = (j * 128) // 512
                for c in range(c0, 2):
                    off = max(0, j * 128 - c * 512)  # within-chunk start
                    sT_ps = ppool_s.tile([128, 512], F32, tag="s")
                    nc.tensor.matmul(
                        sT_ps[:, off:512],
                        xnt_b[:, j * 128 : (j + 1) * 128],
                        y_b[:, c * 512 + off : (c + 1) * 512],
                        start=True,
                        stop=True,
                    )
                    nc.scalar.activation(
                        e_j[:, c * 512 + off : (c + 1) * 512],
                        sT_ps[:, off:512],
                        AF.Exp,
                        scale=SCALE,
                    )
                    if c == c0:
                        # diagonal block: zero the s > t half (keep s <= t)
                        nc.vector.tensor_tensor(
                            out=e_j[:, j * 128 : (j + 1) * 128],
                            in0=e_j[:, j * 128 : (j + 1) * 128],
                            in1=triu_b[:],
                            op=ALU.mult,
                        )

            # ---- G: [pred_u ; colsum] = VW_aug^T eT, then normalize ----
            predt = bigpool.tile([PS, T], F32, tag="pred")
            for n in range(2):
                pu_ps = ppool_o.tile([KA, 512], F32, tag="o")
                for j in range(4 * n + 4):
                    off = max(0, j * 128 - n * 512)
                    nc.tensor.matmul(
                        pu_ps[:, off:512],
                        vw_all[:, j * KA : (j + 1) * KA],
                        et[j][:, n * 512 + off : (n + 1) * 512],
                        start=(j == 0),
                        stop=(j == 4 * n + 3),
                    )
                # pred = pred_u / colsum (+ b_oh via M_vo bias row);
                # 1/colsum = exp(-ln(colsum)) on ScalarE -- DVE reciprocal is
                # an iterative divide (~4.3 cyc/elem) and was the top DVE cost
                lncs = spool.tile([1, 512], F32, tag="rrow")
                nc.scalar.activation(lncs[:], pu_ps[PS : PS + 1, :], AF.Ln)
                rr_b = spool.tile([1, 512], BF16, tag="rrowb")
                nc.scalar.activation(rr_b[:], lncs[:], AF.Exp, scale=-1.0)
                bcr_ps = ppool_t.tile([PS, 512], F32, tag="pt")
                nc.tensor.matmul(
                    bcr_ps[:], ones_row_b[:], rr_b[:], start=True, stop=True
                )
                bcr_sb = scratch.tile([PS, 512], F32, tag="pn")
                nc.vector.tensor_copy(bcr_sb[:], bcr_ps[:])
                nc.vector.tensor_tensor(
                    out=predt[:, n * 512 : (n + 1) * 512],
                    in0=pu_ps[0:PS, :],
                    in1=bcr_sb[:],
                    op=ALU.mult,
                )

            # ---- H: loss partial: sum((pred[:, :-1] - patches[:, 1:])^2) ----
            dd = scratch.tile([PS, T], F32, tag="dd")
            nc.vector.tensor_tensor(
                out=dd[:, 0 : T - 1],
                in0=predt[:, 0 : T - 1],
                in1=xnt_b[0:PS, 1:T],
                op=ALU.subtract,
            )
            nc.scalar.activation(
                dd[:, 0 : T - 1],
                dd[:, 0 : T - 1],
                AF.Square,
                accum_out=lp_all[:, b : b + 1],
            )

        # ---- final: total partial over batches & partitions ----
        lsum = spool.tile([PS, 1], F32)
        nc.vector.reduce_sum(lsum[:], lp_all[:], axis=AX.X)
        tot_ps2 = ppool_s.tile([1, 1], F32, tag="s")
        nc.tensor.matmul(tot_ps2[:], ones_col[0:PS, :], lsum[:], start=True, stop=True)
        out_sb = spool.tile([1, 1], F32)
        nc.vector.tensor_copy(out_sb[:], tot_ps2[:])
        nc.gpsimd.dma_start(out_d.ap()[:], out_sb[:])

    split_excess_waits(nc)
    return nc


_program_cache = {}


def _get_program():
    if "nc" not in _program_cache:
        _program_cache["nc"] = build_program()
    return _program_cache["nc"]


def make_in_maps(x, W_proj, b_proj, W_qkv, b_qkv, W_out, b_out, W_head, b_head):
    import ml_dtypes

    f8 = np.float64
    w_eff = W_proj.astype(f8) @ W_qkv.astype(f8)  # [32, 768]
    b_eff = b_proj.astype(f8) @ W_qkv.astype(f8) + b_qkv.astype(f8)  # [768]
    w_aug = np.concatenate([w_eff, b_eff[None, :]], axis=0)  # [33, 768]
    wq, wk, wv = w_aug[:, 0:D], w_aug[:, D : 2 * D], w_aug[:, 2 * D : 3 * D]
    m_qk = wq @ wk.T  # [33, 33]
    w_oh = W_out.astype(f8) @ W_head.astype(f8)  # [256, 32]
    b_oh = b_out.astype(f8) @ W_head.astype(f8) + b_head.astype(f8)  # [32]
    m_vo = wv @ w_oh  # [33, 32]
    # folding b_oh into the bias row: pred_u' = sum_s (VW + b_oh) eT, so
    # pred_u'/colsum = pred + b_oh exactly.
    m_vo[PS, :] += b_oh
    e_ones = np.zeros((KA, 1), f8)
    e_ones[PS, 0] = 1.0  # selects Xa's ones row -> colsum output column
    m_vo_aug = np.concatenate([m_vo, e_ones], axis=1)  # [33, 33]

    mqk_b = np.ascontiguousarray(m_qk.astype(ml_dtypes.bfloat16))
    mvo_b = np.ascontiguousarray(m_vo_aug.astype(ml_dtypes.bfloat16))

    in_maps = []
    for core in range(N_CORES):
        xs = np.ascontiguousarray(x[core * BPC : (core + 1) * BPC])
        in_maps.append({"x": xs, "m_qk": mqk_b, "m_vo": mvo_b})
    return in_maps


def kernel(**inputs) -> np.ndarray:
    inputs = {k: np.asarray(v) for k, v in inputs.items()}
    nc = _get_program()
    in_maps = make_in_maps(**inputs)
    res = run_bass_kernel_spmd(nc, in_maps, core_ids=list(range(N_CORES)))
    total = sum(float(res.results[i]["loss_partial"][0, 0]) for i in range(N_CORES))
    loss = total / (B * (T - 1) * PS)
    return np.float32(loss)


if __name__ == "__main__":
    rng = np.random.default_rng(0)
    ins = {
        "x": rng.standard_normal((B, L)).astype(np.float32),
        "W_proj": (rng.standard_normal((PS, D)) / math.sqrt(PS)).astype(np.float32),
        "b_proj": np.zeros(D, np.float32),
        "W_qkv": (rng.standard_normal((D, 3 * D)) / math.sqrt(D)).astype(np.float32),
        "b_qkv": np.zeros(3 * D, np.float32),
        "W_out": (rng.standard_normal((D, D)) / math.sqrt(D)).astype(np.float32),
        "b_out": np.zeros(D, np.float32),
        "W_head": (rng.standard_normal((D, PS)) / math.sqrt(D)).astype(np.float32),
        "b_head": np.zeros(PS, np.float32),
    }
    got = kernel(**ins)
    print("kernel loss:", got)


# revision 29
# speedup vs baseline: 2.0770x; 1.0856x over previous
"""Trainium2 Bass kernel for nn_AttentionModel (patch-transformer + MSE loss).

Model (per batch element b of B=32):
    x[b] : [L=32768] --instance-norm--> patches [T=1024, PS=32]
    h    = patches @ W_proj + b_proj                  [T, 256]
    qkv  = h @ W_qkv + b_qkv ;  q,k,v = split(qkv)    [T, 256] each
    attn = softmax(causal(q k^T / 16))                [T, T]
    out  = (attn @ v) @ W_out + b_out                 [T, 256]
    pred = out @ W_head + b_head                      [T, PS]
    loss = mean((pred[:, :-1] - patches[:, 1:])**2)   scalar

Sharding: data-parallel over batch, 4 batch elements per core x 8 cores.
Each core computes a partial sum-of-squares; host combines.

Key algebraic restructure (host-precomputed constants):
    Xa        = [patches^T ; ones]                [33, T] (normalized on-chip)
    M_qk      = Wq_eff Wk_eff^T                   [33, 33]
    M_vo_aug  = [Wv_eff (W_out W_head), e_ones]   [33, 33]
  where Wq_eff = [W_proj W_qkv_q ; b_q_eff] etc. Then:
    scores^T  = Xa^T M_qk Xa      computed as Xa-dot-(M_qk^T Xa), already
                in [s, t] layout, so no PE transposes of the attention
                probabilities are needed.
    VW_aug    = Xa^T M_vo_aug                     [T, 33]
    predu/css = VW_aug^T exp(scores^T/16)         [33, T]
                rows 0..31 = unnormalized pred^T; row 32 = softmax
                denominator (the e_ones column of M_vo_aug sums exp
                columns through the ones row of Xa).
    pred^T    = predu * (1/css) + b_oh
All attention matmuls are bf16 (errors average out in the final scalar
mean); instance-norm, the softmax exp, denominators, and the loss are fp32.
softmax skips the max-subtraction pass: scores/16 ~ N(0,1) and bounded by
|q||k|/16 < ~40, so exp cannot overflow fp32.
sqrt(var) is computed as exp(0.5*ln(var)) so all ScalarE functions come
from one ACT table set (a Sqrt would force ~2.7us table reloads).
"""

import math
import os

import numpy as np

import concourse.bass as bass
import concourse.mybir as mybir
import concourse.tile as tile
from concourse.bass_utils import run_bass_kernel_spmd
from concourse.masks import make_identity, make_upper_triangular
from concourse.vector_clock import ScopedClock

F32 = mybir.dt.float32
BF16 = mybir.dt.bfloat16
AX = mybir.AxisListType
ALU = mybir.AluOpType
AF = mybir.ActivationFunctionType

N_CORES = 8
B = 32
L = 32768
PS = 32
D = 256
T = L // PS  # 1024
BPC = B // N_CORES  # batch elements per core = 4
NT = T // 128  # 8 s-tiles
KA = PS + 1  # augmented contraction dim (extra ones row)
SCALE = 1.0 / math.sqrt(D)  # 1/16


class SplitDrainTileContext(tile.TileContext):
    """TileContext whose final drain splits sem waits across multiple drain
    instructions -- this walrus rejects >1 sync wait per instruction."""

    def _drain_and_barrier(self, tick_clock, wait_clock):
        probe = mybir.InstDrain(name=f"I-{self.nc.next_id()}", ins=[], outs=[])
        probe.engine = mybir.EngineType.SP
        wait_clock.add_sem_waits(probe, ScopedClock({None: tick_clock.global_clock}))
        waits = list(probe.sync_info.on_wait) if probe.sync_info else []
        assert self.sems is not None
        handles = {h.num: h for h in self.sems.allocated().values()}
        if not waits:
            self.nc.sync.drain()
        for w in waits:
            d = self.nc.sync.drain()
            d.wait_op(handles[w.id], w.wait_value, "sem-ge", check=False)
        self.nc.all_engine_barrier()
        popped = self.nc._tile_sem_poison_stack.pop()
        assert popped is self._sem_poison
        self.nc.clear_and_free_semaphores(list(self.sems.allocated().values()))
        self.nc.all_engine_barrier()


def split_excess_waits(nc, max_waits=1):
    """This walrus rejects instructions carrying more than one sync wait.
    Hoist extra waits onto the immediately preceding same-engine
    instruction when that instruction signals nothing (then waiting before
    it cannot starve anyone), else insert a wait-only drain."""
    for f in nc.m.functions:
        for blk in f.blocks:
            insts = list(blk.instructions)
            out = []
            prev_by_engine = {}
            changed = False
            for inst in insts:
                si = inst.sync_info
                waits = list(si.on_wait) if si else []
                if len(waits) > max_waits:
                    changed = True
                    extra, keep = waits[:-max_waits], waits[-max_waits:]
                    remaining = []
                    prev = prev_by_engine.get(str(inst.engine))
                    for w in extra:
                        psi = prev.sync_info if prev is not None else None
                        if prev is not None and (
                            psi is None
                            or (len(psi.on_wait) == 0 and len(psi.on_update) == 0)
                        ):
                            prev.sync_info = mybir.SyncInfo(on_wait=[w], on_update=[])
                            prev = None  # one hoist per predecessor
                        else:
                            remaining.append(w)
                    for w in remaining:
                        dr = mybir.InstDrain(name=f"I-{nc.next_id()}", ins=[], outs=[])
                        dr.engine = inst.engine
                        dr.sync_info = mybir.SyncInfo(on_wait=[w], on_update=[])
                        out.append(dr)
                    inst.sync_info = mybir.SyncInfo(
                        on_wait=keep, on_update=list(si.on_update)
                    )
                out.append(inst)
                prev_by_engine[str(inst.engine)] = inst
            if changed:
                blk.instructions = out


def build_program():
    nc = bass.Bass("TRN2", target_bir_lowering=False, debug=False, num_devices=N_CORES)

    x_d = nc.dram_tensor("x", [BPC, L], F32, kind="ExternalInput")
    mqk_d = nc.dram_tensor("m_qk", [KA, KA], BF16, kind="ExternalInput")
    mvo_d = nc.dram_tensor("m_vo", [KA, KA], BF16, kind="ExternalInput")
    out_d = nc.dram_tensor("loss_partial", [1, 1], F32, kind="ExternalOutput")

    from contextlib import ExitStack

    with SplitDrainTileContext(nc) as tc, ExitStack() as ctx:
        cpool = ctx.enter_context(tc.tile_pool(name="consts", bufs=1))
        ppool_s = ctx.enter_context(tc.tile_pool(name="psum_s", bufs=4, space="PSUM"))
        ppool_o = ctx.enter_context(tc.tile_pool(name="psum_o", bufs=2, space="PSUM"))
        ppool_t = ctx.enter_context(tc.tile_pool(name="psum_t", bufs=2, space="PSUM"))
        xpool = ctx.enter_context(tc.tile_pool(name="xc", bufs=3))
        spool = ctx.enter_context(tc.tile_pool(name="small", bufs=6))
        bigpool = ctx.enter_context(tc.tile_pool(name="big", bufs=3))
        epool = ctx.enter_context(tc.tile_pool(name="et", bufs=18))
        scratch = ctx.enter_context(tc.tile_pool(name="scratch", bufs=4))

        # ---- constants ----
        ident_f = cpool.tile([128, 128], F32)
        make_identity(nc, ident_f[:])
        triu_b = cpool.tile([128, 128], BF16)
        make_upper_triangular(nc, triu_b[:], val=1.0, diag=True)
        ones_col = cpool.tile([128, 1], F32)
        nc.vector.memset(ones_col[:], 1.0)
        ones_row = cpool.tile([1, PS], F32)
        nc.vector.memset(ones_row[:], 1.0)
        ones_row_b = cpool.tile([1, PS], BF16)
        nc.vector.memset(ones_row_b[:], 1.0)

        mqk = cpool.tile([KA, KA], BF16)
        nc.gpsimd.dma_start(mqk[:], mqk_d.ap()[:])
        mvo = cpool.tile([KA, KA], BF16)
        nc.gpsimd.dma_start(mvo[:], mvo_d.ap()[:])

        lp_all = cpool.tile([PS, BPC], F32)  # per-batch loss partials

        for b in range(BPC):
            # ---- A: load x[b] contiguously as [128, 256] ----
            xc = xpool.tile([128, L // 128], F32)
            # partition u, free (k, ps) <- x[b, (128k + u)*32 + ps]: each
            # partition gets 8 tokens at stride 4KB; transposing column block
            # k then yields tokens [128k, 128k+128) contiguously.
            nc.gpsimd.dma_start(
                xc[:].rearrange("u (k ps) -> u k ps", ps=PS),
                x_d.ap()[b].rearrange("(k u ps) -> u k ps", u=128, ps=PS),
            )

            # ---- B: instance-norm stats ----
            sums = spool.tile([128, 2], F32)
            nc.vector.reduce_sum(sums[:, 0:1], xc[:], axis=AX.X)
            sq_scr = scratch.tile([128, L // 128], F32)
            nc.vector.tensor_tensor(out=sq_scr[:], in0=xc[:], in1=xc[:], op=ALU.mult)
            nc.vector.reduce_sum(sums[:, 1:2], sq_scr[:], axis=AX.X)
            tot_ps = ppool_s.tile([1, 2], F32, tag="s")
            nc.tensor.matmul(tot_ps[:], ones_col[:], sums[:], start=True, stop=True)
            tot = spool.tile([1, 2], F32)
            nc.vector.tensor_copy(tot[:], tot_ps[:])

            # sc = [mean, s*m, ssq-s*m, ln, std, std+eps, rstd, -m*rstd, -mean]
            sc = spool.tile([1, 9], F32)
            nc.scalar.mul(sc[:, 0:1], tot[:, 0:1], 1.0 / L)  # mean
            nc.vector.tensor_tensor(
                out=sc[:, 1:2], in0=tot[:, 0:1], in1=sc[:, 0:1], op=ALU.mult
            )
            nc.vector.tensor_tensor(
                out=sc[:, 2:3], in0=tot[:, 1:2], in1=sc[:, 1:2], op=ALU.subtract
            )
            nc.scalar.activation(sc[:, 3:4], sc[:, 2:3], AF.Ln, scale=1.0 / (L - 1))
            nc.scalar.activation(sc[:, 4:5], sc[:, 3:4], AF.Exp, scale=0.5)  # std
            nc.vector.tensor_scalar_add(sc[:, 5:6], sc[:, 4:5], 1e-5)
            nc.vector.reciprocal(sc[:, 6:7], sc[:, 5:6])  # rstd
            nc.scalar.mul(sc[:, 8:9], sc[:, 0:1], -1.0)  # -mean
            nc.vector.tensor_tensor(
                out=sc[:, 7:8], in0=sc[:, 8:9], in1=sc[:, 6:7], op=ALU.mult
            )  # -mean*rstd

            # broadcast [rstd, -mean*rstd] to 32 partitions via rank-1 matmul
            bc_ps = ppool_s.tile([PS, 2], F32, tag="s")
            nc.tensor.matmul(bc_ps[:], ones_row[:], sc[:, 6:8], start=True, stop=True)
            bc = spool.tile([PS, 2], F32)
            nc.vector.tensor_copy(bc[:], bc_ps[:])

            # ---- C: transpose x into patch-major Xa [33, 1024], normalized;
            # row 32 is the constant-1 augmentation row ----
            xnt_b = bigpool.tile([KA, T], BF16, tag="xntb")
            nc.gpsimd.memset(xnt_b[PS : PS + 1, :], 1.0)
            for r in range(2):
                xt_ps = ppool_t.tile([PS, 512], F32, tag="pt")
                for c in range(4):
                    k = 4 * r + c
                    nc.tensor.transpose(
                        xt_ps[:, c * 128 : (c + 1) * 128],
                        xc[:, k * PS : (k + 1) * PS],
                        ident_f[:],
                    )
                # xt_ps[ps, c*128+u] = token 128*(4r+c)+u elem ps: dense write
                nc.vector.tensor_scalar(
                    out=xnt_b[0:PS, r * 512 : (r + 1) * 512],
                    in0=xt_ps[:],
                    scalar1=bc[:, 0:1],
                    scalar2=bc[:, 1:2],
                    op0=ALU.mult,
                    op1=ALU.add,
                )
            # ---- D: Y = M_qk^T Xa  [33, 1024] bf16 ----
            y_b = bigpool.tile([KA, T], BF16, tag="y")
            for n in range(2):
                y_ps = ppool_s.tile([KA, 512], F32, tag="s")
                nc.tensor.matmul(
                    y_ps[:],
                    mqk[:],
                    xnt_b[:, n * 512 : (n + 1) * 512],
                    start=True,
                    stop=True,
                )
                nc.vector.tensor_copy(y_b[:, n * 512 : (n + 1) * 512], y_ps[:])

            # ---- E: VW_aug = Xa^T M_vo_aug  [8][128, 33] bf16 ----
            vw_all = bigpool.tile([128, NT * KA], BF16, tag="vw")
            for h in range(2):
                vw_ps = ppool_s.tile([128, 4 * KA], F32, tag="s")
                for jj in range(4):
                    j = 4 * h + jj
                    nc.tensor.matmul(
                        vw_ps[:, jj * KA : (jj + 1) * KA],
                        xnt_b[:, j * 128 : (j + 1) * 128],
                        mvo[:],
                        start=True,
                        stop=True,
                    )
                nc.vector.tensor_copy(
                    vw_all[:, h * 4 * KA : (h + 1) * 4 * KA], vw_ps[:]
                )

            # ---- F+G interleaved per t-half: produce the eT chunks a
            # t-half needs, run its PV/pred accumulation, and overlap its
            # normalization epilogue with the next half's eT production ----
            et = {}
            predt = bigpool.tile([PS, T], F32, tag="pred")
            for n in range(2):
                for j in range(4 * n + 4):
                    if j not in et:
                        et[j] = epool.tile(
                            [128, T], BF16, tag="et", name=f"et_{b}_{j}"
                        )
                    e_j = et[j]
                    c = n  # chunk index == t-half
                    c0 = (j * 128) // 512
                    if c < c0:
                        continue  # entirely non-causal for this half
                    off = max(0, j * 128 - c * 512)  # within-chunk start
                    sT_ps = ppool_s.tile([128, 512], F32, tag="s")
                    nc.tensor.matmul(
                        sT_ps[:, off:512],
                        xnt_b[:, j * 128 : (j + 1) * 128],
                        y_b[:, c * 512 + off : (c + 1) * 512],
                        start=True,
                        stop=True,
                    )
                    nc.scalar.activation(
                        e_j[:, c * 512 + off : (c + 1) * 512],
                        sT_ps[:, off:512],
                        AF.Exp,
                        scale=SCALE,
                    )
                    if c == c0:
                        # diagonal block: zero the s > t half (keep s <= t)
                        nc.vector.tensor_tensor(
                            out=e_j[:, j * 128 : (j + 1) * 128],
                            in0=e_j[:, j * 128 : (j + 1) * 128],
                            in1=triu_b[:],
                            op=ALU.mult,
                        )
                pu_ps = ppool_o.tile([KA, 512], F32, tag="o")
                for j in range(4 * n + 4):
                    off = max(0, j * 128 - n * 512)
                    nc.tensor.matmul(
                        pu_ps[:, off:512],
                        vw_all[:, j * KA : (j + 1) * KA],
                        et[j][:, n * 512 + off : (n + 1) * 512],
                        start=(j == 0),
                        stop=(j == 4 * n + 3),
                    )
                # pred = pred_u / colsum (+ b_oh via M_vo bias row);
                # 1/colsum = exp(-ln(colsum)) on ScalarE -- DVE reciprocal is
                # an iterative divide (~4.3 cyc/elem) and was the top DVE cost
                lncs = spool.tile([1, 512], F32, tag="rrow")
                nc.scalar.activation(lncs[:], pu_ps[PS : PS + 1, :], AF.Ln)
                rr_b = spool.tile([1, 512], BF16, tag="rrowb")
                nc.scalar.activation(rr_b[:], lncs[:], AF.Exp, scale=-1.0)
                bcr_ps = ppool_t.tile([PS, 512], F32, tag="pt")
                nc.tensor.matmul(
                    bcr_ps[:], ones_row_b[:], rr_b[:], start=True, stop=True
                )
                bcr_sb = scratch.tile([PS, 512], F32, tag="pn")
                nc.vector.tensor_copy(bcr_sb[:], bcr_ps[:])
                nc.vector.tensor_tensor(
                    out=predt[:, n * 512 : (n + 1) * 512],
                    in0=pu_ps[0:PS, :],
                    in1=bcr_sb[:],
                    op=ALU.mult,
                )

            # ---- H: loss partial: sum((pred[:, :-1] - patches[:, 1:])^2) ----
            dd = scratch.tile([PS, T], F32, tag="dd")
            nc.vector.tensor_tensor(
                out=dd[:, 0 : T - 1],
                in0=predt[:, 0 : T - 1],
                in1=xnt_b[0:PS, 1:T],
                op=ALU.subtract,
            )
            nc.scalar.activation(
                dd[:, 0 : T - 1],
                dd[:, 0 : T - 1],
                AF.Square,
                accum_out=lp_all[:, b : b + 1],
            )

        # ---- final: total partial over batches & partitions ----
        lsum = spool.tile([PS, 1], F32)
        nc.vector.reduce_sum(lsum[:], lp_all[:], axis=AX.X)
        tot_ps2 = ppool_s.tile([1, 1], F32, tag="s")
        nc.tensor.matmul(tot_ps2[:], ones_col[0:PS, :], lsum[:], start=True, stop=True)
        out_sb = spool.tile([1, 1], F32)
        nc.vector.tensor_copy(out_sb[:], tot_ps2[:])
        nc.gpsimd.dma_start(out_d.ap()[:], out_sb[:])

    split_excess_waits(nc)
    return nc


_program_cache = {}


def _get_program():
    if "nc" not in _program_cache:
        _program_cache["nc"] = build_program()
    return _program_cache["nc"]


def make_in_maps(x, W_proj, b_proj, W_qkv, b_qkv, W_out, b_out, W_head, b_head):
    import ml_dtypes

    f8 = np.float64
    w_eff = W_proj.astype(f8) @ W_qkv.astype(f8)  # [32, 768]
    b_eff = b_proj.astype(f8) @ W_qkv.astype(f8) + b_qkv.astype(f8)  # [768]
    w_aug = np.concatenate([w_eff, b_eff[None, :]], axis=0)  # [33, 768]
    wq, wk, wv = w_aug[:, 0:D], w_aug[:, D : 2 * D], w_aug[:, 2 * D : 3 * D]
    m_qk = wq @ wk.T  # [33, 33]
    w_oh = W_out.astype(f8) @ W_head.astype(f8)  # [256, 32]
    b_oh = b_out.astype(f8) @ W_head.astype(f8) + b_head.astype(f8)  # [32]
    m_vo = wv @ w_oh  # [33, 32]
    # folding b_oh into the bias row: pred_u' = sum_s (VW + b_oh) eT, so
    # pred_u'/colsum = pred + b_oh exactly.
    m_vo[PS, :] += b_oh
    e_ones = np.zeros((KA, 1), f8)
    e_ones[PS, 0] = 1.0  # selects Xa's ones row -> colsum output column
    m_vo_aug = np.concatenate([m_vo, e_ones], axis=1)  # [33, 33]

    mqk_b = np.ascontiguousarray(m_qk.astype(ml_dtypes.bfloat16))
    mvo_b = np.ascontiguousarray(m_vo_aug.astype(ml_dtypes.bfloat16))

    in_maps = []
    for core in range(N_CORES):
        xs = np.ascontiguousarray(x[core * BPC : (core + 1) * BPC])
        in_maps.append({"x": xs, "m_qk": mqk_b, "m_vo": mvo_b})
    return in_maps


def kernel(**inputs) -> np.ndarray:
    inputs = {k: np.asarray(v) for k, v in inputs.items()}
    nc = _get_program()
    in_maps = make_in_maps(**inputs)
    res = run_bass_kernel_spmd(nc, in_maps, core_ids=list(range(N_CORES)))
    total = sum(float(res.results[i]["loss_partial"][0, 0]) for i in range(N_CORES))
    loss = total / (B * (T - 1) * PS)
    return np.float32(loss)


if __name__ == "__main__":
    rng = np.random.default_rng(0)
    ins = {
        "x": rng.standard_normal((B, L)).astype(np.float32),
        "W_proj": (rng.standard_normal((PS, D)) / math.sqrt(PS)).astype(np.float32),
        "b_proj": np.zeros(D, np.float32),
        "W_qkv": (rng.standard_normal((D, 3 * D)) / math.sqrt(D)).astype(np.float32),
        "b_qkv": np.zeros(3 * D, np.float32),
        "W_out": (rng.standard_normal((D, D)) / math.sqrt(D)).astype(np.float32),
        "b_out": np.zeros(D, np.float32),
        "W_head": (rng.standard_normal((D, PS)) / math.sqrt(D)).astype(np.float32),
        "b_head": np.zeros(PS, np.float32),
    }
    got = kernel(**ins)
    print("kernel loss:", got)


# revision 30
# speedup vs baseline: 2.1171x; 1.0193x over previous
"""Trainium2 Bass kernel for nn_AttentionModel (patch-transformer + MSE loss).

Model (per batch element b of B=32):
    x[b] : [L=32768] --instance-norm--> patches [T=1024, PS=32]
    h    = patches @ W_proj + b_proj                  [T, 256]
    qkv  = h @ W_qkv + b_qkv ;  q,k,v = split(qkv)    [T, 256] each
    attn = softmax(causal(q k^T / 16))                [T, T]
    out  = (attn @ v) @ W_out + b_out                 [T, 256]
    pred = out @ W_head + b_head                      [T, PS]
    loss = mean((pred[:, :-1] - patches[:, 1:])**2)   scalar

Sharding: data-parallel over batch, 4 batch elements per core x 8 cores.
Each core computes a partial sum-of-squares; host combines.

Key algebraic restructure (host-precomputed constants):
    Xa        = [patches^T ; ones]                [33, T] (normalized on-chip)
    M_qk      = Wq_eff Wk_eff^T                   [33, 33]
    M_vo_aug  = [Wv_eff (W_out W_head), e_ones]   [33, 33]
  where Wq_eff = [W_proj W_qkv_q ; b_q_eff] etc. Then:
    scores^T  = Xa^T M_qk Xa      computed as Xa-dot-(M_qk^T Xa), already
                in [s, t] layout, so no PE transposes of the attention
                probabilities are needed.
    VW_aug    = Xa^T M_vo_aug                     [T, 33]
    predu/css = VW_aug^T exp(scores^T/16)         [33, T]
                rows 0..31 = unnormalized pred^T; row 32 = softmax
                denominator (the e_ones column of M_vo_aug sums exp
                columns through the ones row of Xa).
    pred^T    = predu * (1/css) + b_oh
All attention matmuls are bf16 (errors average out in the final scalar
mean); instance-norm, the softmax exp, denominators, and the loss are fp32.
softmax skips the max-subtraction pass: scores/16 ~ N(0,1) and bounded by
|q||k|/16 < ~40, so exp cannot overflow fp32.
sqrt(var) is computed as exp(0.5*ln(var)) so all ScalarE functions come
from one ACT table set (a Sqrt would force ~2.7us table reloads).
"""

import math
import os

import numpy as np

import concourse.bass as bass
import concourse.mybir as mybir
import concourse.tile as tile
from concourse.bass_utils import run_bass_kernel_spmd
from concourse.masks import make_identity, make_upper_triangular
from concourse.vector_clock import ScopedClock

F32 = mybir.dt.float32
BF16 = mybir.dt.bfloat16
AX = mybir.AxisListType
ALU = mybir.AluOpType
AF = mybir.ActivationFunctionType

N_CORES = 8
B = 32
L = 32768
PS = 32
D = 256
T = L // PS  # 1024
BPC = B // N_CORES  # batch elements per core = 4
NT = T // 128  # 8 s-tiles
KA = PS + 1  # augmented contraction dim (extra ones row)
SCALE = 1.0 / math.sqrt(D)  # 1/16


class SplitDrainTileContext(tile.TileContext):
    """TileContext whose final drain splits sem waits across multiple drain
    instructions -- this walrus rejects >1 sync wait per instruction."""

    def _drain_and_barrier(self, tick_clock, wait_clock):
        probe = mybir.InstDrain(name=f"I-{self.nc.next_id()}", ins=[], outs=[])
        probe.engine = mybir.EngineType.SP
        wait_clock.add_sem_waits(probe, ScopedClock({None: tick_clock.global_clock}))
        waits = list(probe.sync_info.on_wait) if probe.sync_info else []
        assert self.sems is not None
        handles = {h.num: h for h in self.sems.allocated().values()}
        if not waits:
            self.nc.sync.drain()
        for w in waits:
            d = self.nc.sync.drain()
            d.wait_op(handles[w.id], w.wait_value, "sem-ge", check=False)
        self.nc.all_engine_barrier()
        popped = self.nc._tile_sem_poison_stack.pop()
        assert popped is self._sem_poison
        self.nc.clear_and_free_semaphores(list(self.sems.allocated().values()))
        self.nc.all_engine_barrier()


def split_excess_waits(nc, max_waits=1):
    """This walrus rejects instructions carrying more than one sync wait.
    Hoist extra waits onto the immediately preceding same-engine
    instruction when that instruction signals nothing (then waiting before
    it cannot starve anyone), else insert a wait-only drain."""
    for f in nc.m.functions:
        for blk in f.blocks:
            insts = list(blk.instructions)
            out = []
            prev_by_engine = {}
            changed = False
            for inst in insts:
                si = inst.sync_info
                waits = list(si.on_wait) if si else []
                if len(waits) > max_waits:
                    changed = True
                    extra, keep = waits[:-max_waits], waits[-max_waits:]
                    remaining = []
                    prev = prev_by_engine.get(str(inst.engine))
                    for w in extra:
                        psi = prev.sync_info if prev is not None else None
                        if prev is not None and (
                            psi is None
                            or (len(psi.on_wait) == 0 and len(psi.on_update) == 0)
                        ):
                            prev.sync_info = mybir.SyncInfo(on_wait=[w], on_update=[])
                            prev = None  # one hoist per predecessor
                        else:
                            remaining.append(w)
                    for w in remaining:
                        dr = mybir.InstDrain(name=f"I-{nc.next_id()}", ins=[], outs=[])
                        dr.engine = inst.engine
                        dr.sync_info = mybir.SyncInfo(on_wait=[w], on_update=[])
                        out.append(dr)
                    inst.sync_info = mybir.SyncInfo(
                        on_wait=keep, on_update=list(si.on_update)
                    )
                out.append(inst)
                prev_by_engine[str(inst.engine)] = inst
            if changed:
                blk.instructions = out


def dedupe_ldweights(nc):
    """Drop an InstLdweights whose operand AP is byte-identical to the
    immediately preceding PE instruction's InstLdweights (no other PE
    instruction between them) -- the stationary operand is still loaded.
    Only legal when the elided load carries no sync actions."""
    import json as _json

    for f in nc.m.functions:
        for blk in f.blocks:
            insts = list(blk.instructions)
            out = []
            last_pe_ldw_key = None
            changed = False
            for inst in insts:
                if str(inst.engine) != "EngineType.PE":
                    out.append(inst)
                    continue
                tname = type(inst).__name__
                if tname == "InstLdweights":
                    si = inst.sync_info
                    has_sync = si and (len(si.on_wait) or len(si.on_update))
                    try:
                        key = str(inst.ins[0])
                    except Exception:
                        key = None
                    if (
                        key is not None
                        and key == last_pe_ldw_key
                        and not has_sync
                    ):
                        changed = True
                        continue  # elide duplicate load
                    last_pe_ldw_key = key
                    out.append(inst)
                else:
                    if tname == "InstMatmult":
                        # transpose-mode matmuls reload the array themselves
                        if getattr(inst, "is_transpose", None):
                            last_pe_ldw_key = None
                    else:
                        last_pe_ldw_key = None
                    out.append(inst)
            if changed:
                blk.instructions = out


def build_program():
    nc = bass.Bass("TRN2", target_bir_lowering=False, debug=False, num_devices=N_CORES)

    x_d = nc.dram_tensor("x", [BPC, L], F32, kind="ExternalInput")
    mqk_d = nc.dram_tensor("m_qk", [KA, KA], BF16, kind="ExternalInput")
    mvo_d = nc.dram_tensor("m_vo", [KA, KA], BF16, kind="ExternalInput")
    out_d = nc.dram_tensor("loss_partial", [1, 1], F32, kind="ExternalOutput")

    from contextlib import ExitStack

    with SplitDrainTileContext(nc) as tc, ExitStack() as ctx:
        cpool = ctx.enter_context(tc.tile_pool(name="consts", bufs=1))
        ppool_s = ctx.enter_context(tc.tile_pool(name="psum_s", bufs=4, space="PSUM"))
        ppool_o = ctx.enter_context(tc.tile_pool(name="psum_o", bufs=2, space="PSUM"))
        ppool_t = ctx.enter_context(tc.tile_pool(name="psum_t", bufs=2, space="PSUM"))
        xpool = ctx.enter_context(tc.tile_pool(name="xc", bufs=3))
        spool = ctx.enter_context(tc.tile_pool(name="small", bufs=6))
        bigpool = ctx.enter_context(tc.tile_pool(name="big", bufs=3))
        epool = ctx.enter_context(tc.tile_pool(name="et", bufs=18))
        scratch = ctx.enter_context(tc.tile_pool(name="scratch", bufs=4))

        # ---- constants ----
        ident_f = cpool.tile([128, 128], F32)
        make_identity(nc, ident_f[:])
        triu_b = cpool.tile([128, 128], BF16)
        make_upper_triangular(nc, triu_b[:], val=1.0, diag=True)
        ones_col = cpool.tile([128, 1], F32)
        nc.vector.memset(ones_col[:], 1.0)
        ones_row = cpool.tile([1, PS], F32)
        nc.vector.memset(ones_row[:], 1.0)
        ones_row_b = cpool.tile([1, PS], BF16)
        nc.vector.memset(ones_row_b[:], 1.0)

        mqk = cpool.tile([KA, KA], BF16)
        nc.gpsimd.dma_start(mqk[:], mqk_d.ap()[:])
        mvo = cpool.tile([KA, KA], BF16)
        nc.gpsimd.dma_start(mvo[:], mvo_d.ap()[:])

        lp_all = cpool.tile([PS, BPC], F32)  # per-batch loss partials

        for b in range(BPC):
            # ---- A: load x[b] contiguously as [128, 256] ----
            xc = xpool.tile([128, L // 128], F32)
            # partition u, free (k, ps) <- x[b, (128k + u)*32 + ps]: each
            # partition gets 8 tokens at stride 4KB; transposing column block
            # k then yields tokens [128k, 128k+128) contiguously.
            nc.sync.dma_start(
                xc[:].rearrange("u (k ps) -> u k ps", ps=PS),
                x_d.ap()[b].rearrange("(k u ps) -> u k ps", u=128, ps=PS),
            )

            # ---- B: instance-norm stats ----
            sums = spool.tile([128, 2], F32)
            nc.vector.reduce_sum(sums[:, 0:1], xc[:], axis=AX.X)
            sq_scr = scratch.tile([128, L // 128], F32)
            nc.vector.tensor_tensor(out=sq_scr[:], in0=xc[:], in1=xc[:], op=ALU.mult)
            nc.vector.reduce_sum(sums[:, 1:2], sq_scr[:], axis=AX.X)
            tot_ps = ppool_s.tile([1, 2], F32, tag="s")
            nc.tensor.matmul(tot_ps[:], ones_col[:], sums[:], start=True, stop=True)
            tot = spool.tile([1, 2], F32)
            nc.vector.tensor_copy(tot[:], tot_ps[:])

            # sc = [mean, s*m, ssq-s*m, ln, std, std+eps, rstd, -m*rstd, -mean]
            sc = spool.tile([1, 9], F32)
            nc.scalar.mul(sc[:, 0:1], tot[:, 0:1], 1.0 / L)  # mean
            nc.vector.tensor_tensor(
                out=sc[:, 1:2], in0=tot[:, 0:1], in1=sc[:, 0:1], op=ALU.mult
            )
            nc.vector.tensor_tensor(
                out=sc[:, 2:3], in0=tot[:, 1:2], in1=sc[:, 1:2], op=ALU.subtract
            )
            nc.scalar.activation(sc[:, 3:4], sc[:, 2:3], AF.Ln, scale=1.0 / (L - 1))
            nc.scalar.activation(sc[:, 4:5], sc[:, 3:4], AF.Exp, scale=0.5)  # std
            nc.vector.tensor_scalar_add(sc[:, 5:6], sc[:, 4:5], 1e-5)
            nc.vector.reciprocal(sc[:, 6:7], sc[:, 5:6])  # rstd
            nc.scalar.mul(sc[:, 8:9], sc[:, 0:1], -1.0)  # -mean
            nc.vector.tensor_tensor(
                out=sc[:, 7:8], in0=sc[:, 8:9], in1=sc[:, 6:7], op=ALU.mult
            )  # -mean*rstd

            # broadcast [rstd, -mean*rstd] to 32 partitions via rank-1 matmul
            bc_ps = ppool_s.tile([PS, 2], F32, tag="s")
            nc.tensor.matmul(bc_ps[:], ones_row[:], sc[:, 6:8], start=True, stop=True)
            bc = spool.tile([PS, 2], F32)
            nc.vector.tensor_copy(bc[:], bc_ps[:])

            # ---- C: transpose x into patch-major Xa [33, 1024], normalized;
            # row 32 is the constant-1 augmentation row ----
            xnt_b = bigpool.tile([KA, T], BF16, tag="xntb")
            nc.gpsimd.memset(xnt_b[PS : PS + 1, :], 1.0)
            for r in range(2):
                xt_ps = ppool_t.tile([PS, 512], F32, tag="pt")
                for c in range(4):
                    k = 4 * r + c
                    nc.tensor.transpose(
                        xt_ps[:, c * 128 : (c + 1) * 128],
                        xc[:, k * PS : (k + 1) * PS],
                        ident_f[:],
                    )
                # xt_ps[ps, c*128+u] = token 128*(4r+c)+u elem ps: dense write
                nc.vector.tensor_scalar(
                    out=xnt_b[0:PS, r * 512 : (r + 1) * 512],
                    in0=xt_ps[:],
                    scalar1=bc[:, 0:1],
                    scalar2=bc[:, 1:2],
                    op0=ALU.mult,
                    op1=ALU.add,
                )
            # ---- D: Y = M_qk^T Xa  [33, 1024] bf16 ----
            y_b = bigpool.tile([KA, T], BF16, tag="y")
            for n in range(2):
                y_ps = ppool_s.tile([KA, 512], F32, tag="s")
                nc.tensor.matmul(
                    y_ps[:],
                    mqk[:],
                    xnt_b[:, n * 512 : (n + 1) * 512],
                    start=True,
                    stop=True,
                )
                nc.vector.tensor_copy(y_b[:, n * 512 : (n + 1) * 512], y_ps[:])

            # ---- E: VW_aug = Xa^T M_vo_aug  [8][128, 33] bf16 ----
            vw_all = bigpool.tile([128, NT * KA], BF16, tag="vw")
            for h in range(2):
                vw_ps = ppool_s.tile([128, 4 * KA], F32, tag="s")
                for jj in range(4):
                    j = 4 * h + jj
                    nc.tensor.matmul(
                        vw_ps[:, jj * KA : (jj + 1) * KA],
                        xnt_b[:, j * 128 : (j + 1) * 128],
                        mvo[:],
                        start=True,
                        stop=True,
                    )
                nc.vector.tensor_copy(
                    vw_all[:, h * 4 * KA : (h + 1) * 4 * KA], vw_ps[:]
                )

            # ---- F+G interleaved per t-half: produce the eT chunks a
            # t-half needs, run its PV/pred accumulation, and overlap its
            # normalization epilogue with the next half's eT production ----
            et = {}
            predt = bigpool.tile([PS, T], F32, tag="pred")
            for n in range(2):
                for j in range(4 * n + 4):
                    if j not in et:
                        et[j] = epool.tile(
                            [128, T], BF16, tag="et", name=f"et_{b}_{j}"
                        )
                    e_j = et[j]
                    c = n  # chunk index == t-half
                    c0 = (j * 128) // 512
                    if c < c0:
                        continue  # entirely non-causal for this half
                    off = max(0, j * 128 - c * 512)  # within-chunk start
                    sT_ps = ppool_s.tile([128, 512], F32, tag="s")
                    nc.tensor.matmul(
                        sT_ps[:, off:512],
                        xnt_b[:, j * 128 : (j + 1) * 128],
                        y_b[:, c * 512 + off : (c + 1) * 512],
                        start=True,
                        stop=True,
                    )
                    nc.scalar.activation(
                        e_j[:, c * 512 + off : (c + 1) * 512],
                        sT_ps[:, off:512],
                        AF.Exp,
                        scale=SCALE,
                    )
                    if c == c0:
                        # diagonal block: zero the s > t half (keep s <= t)
                        nc.vector.tensor_tensor(
                            out=e_j[:, j * 128 : (j + 1) * 128],
                            in0=e_j[:, j * 128 : (j + 1) * 128],
                            in1=triu_b[:],
                            op=ALU.mult,
                        )
                pu_ps = ppool_o.tile([KA, 512], F32, tag="o")
                for j in range(4 * n + 4):
                    off = max(0, j * 128 - n * 512)
                    nc.tensor.matmul(
                        pu_ps[:, off:512],
                        vw_all[:, j * KA : (j + 1) * KA],
                        et[j][:, n * 512 + off : (n + 1) * 512],
                        start=(j == 0),
                        stop=(j == 4 * n + 3),
                    )
                # pred = pred_u / colsum (+ b_oh via M_vo bias row);
                # 1/colsum = exp(-ln(colsum)) on ScalarE -- DVE reciprocal is
                # an iterative divide (~4.3 cyc/elem) and was the top DVE cost
                lncs = spool.tile([1, 512], F32, tag="rrow")
                nc.scalar.activation(lncs[:], pu_ps[PS : PS + 1, :], AF.Ln)
                rr_b = spool.tile([1, 512], BF16, tag="rrowb")
                nc.scalar.activation(rr_b[:], lncs[:], AF.Exp, scale=-1.0)
                bcr_ps = ppool_t.tile([PS, 512], F32, tag="pt")
                nc.tensor.matmul(
                    bcr_ps[:], ones_row_b[:], rr_b[:], start=True, stop=True
                )
                bcr_sb = scratch.tile([PS, 512], F32, tag="pn")
                nc.vector.tensor_copy(bcr_sb[:], bcr_ps[:])
                nc.vector.tensor_tensor(
                    out=predt[:, n * 512 : (n + 1) * 512],
                    in0=pu_ps[0:PS, :],
                    in1=bcr_sb[:],
                    op=ALU.mult,
                )

            # ---- H: loss partial: sum((pred[:, :-1] - patches[:, 1:])^2) ----
            dd = scratch.tile([PS, T], F32, tag="dd")
            nc.vector.tensor_tensor(
                out=dd[:, 0 : T - 1],
                in0=predt[:, 0 : T - 1],
                in1=xnt_b[0:PS, 1:T],
                op=ALU.subtract,
            )
            nc.scalar.activation(
                dd[:, 0 : T - 1],
                dd[:, 0 : T - 1],
                AF.Square,
                accum_out=lp_all[:, b : b + 1],
            )

        # ---- final: total partial over batches & partitions ----
        lsum = spool.tile([PS, 1], F32)
        nc.vector.reduce_sum(lsum[:], lp_all[:], axis=AX.X)
        tot_ps2 = ppool_s.tile([1, 1], F32, tag="s")
        nc.tensor.matmul(tot_ps2[:], ones_col[0:PS, :], lsum[:], start=True, stop=True)
        out_sb = spool.tile([1, 1], F32)
        nc.vector.tensor_copy(out_sb[:], tot_ps2[:])
        nc.gpsimd.dma_start(out_d.ap()[:], out_sb[:])

    split_excess_waits(nc)
    dedupe_ldweights(nc)
    return nc


_program_cache = {}


def _get_program():
    if "nc" not in _program_cache:
        _program_cache["nc"] = build_program()
    return _program_cache["nc"]


def make_in_maps(x, W_proj, b_proj, W_qkv, b_qkv, W_out, b_out, W_head, b_head):
    import ml_dtypes

    f8 = np.float64
    w_eff = W_proj.astype(f8) @ W_qkv.astype(f8)  # [32, 768]
    b_eff = b_proj.astype(f8) @ W_qkv.astype(f8) + b_qkv.astype(f8)  # [768]
    w_aug = np.concatenate([w_eff, b_eff[None, :]], axis=0)  # [33, 768]
    wq, wk, wv = w_aug[:, 0:D], w_aug[:, D : 2 * D], w_aug[:, 2 * D : 3 * D]
    m_qk = wq @ wk.T  # [33, 33]
    w_oh = W_out.astype(f8) @ W_head.astype(f8)  # [256, 32]
    b_oh = b_out.astype(f8) @ W_head.astype(f8) + b_head.astype(f8)  # [32]
    m_vo = wv @ w_oh  # [33, 32]
    # folding b_oh into the bias row: pred_u' = sum_s (VW + b_oh) eT, so
    # pred_u'/colsum = pred + b_oh exactly.
    m_vo[PS, :] += b_oh
    e_ones = np.zeros((KA, 1), f8)
    e_ones[PS, 0] = 1.0  # selects Xa's ones row -> colsum output column
    m_vo_aug = np.concatenate([m_vo, e_ones], axis=1)  # [33, 33]

    mqk_b = np.ascontiguousarray(m_qk.astype(ml_dtypes.bfloat16))
    mvo_b = np.ascontiguousarray(m_vo_aug.astype(ml_dtypes.bfloat16))

    in_maps = []
    for core in range(N_CORES):
        xs = np.ascontiguousarray(x[core * BPC : (core + 1) * BPC])
        in_maps.append({"x": xs, "m_qk": mqk_b, "m_vo": mvo_b})
    return in_maps


def kernel(**inputs) -> np.ndarray:
    inputs = {k: np.asarray(v) for k, v in inputs.items()}
    nc = _get_program()
    in_maps = make_in_maps(**inputs)
    res = run_bass_kernel_spmd(nc, in_maps, core_ids=list(range(N_CORES)))
    total = sum(float(res.results[i]["loss_partial"][0, 0]) for i in range(N_CORES))
    loss = total / (B * (T - 1) * PS)
    return np.float32(loss)


if __name__ == "__main__":
    rng = np.random.default_rng(0)
    ins = {
        "x": rng.standard_normal((B, L)).astype(np.float32),
        "W_proj": (rng.standard_normal((PS, D)) / math.sqrt(PS)).astype(np.float32),
        "b_proj": np.zeros(D, np.float32),
        "W_qkv": (rng.standard_normal((D, 3 * D)) / math.sqrt(D)).astype(np.float32),
        "b_qkv": np.zeros(3 * D, np.float32),
        "W_out": (rng.standard_normal((D, D)) / math.sqrt(D)).astype(np.float32),
        "b_out": np.zeros(D, np.float32),
        "W_head": (rng.standard_normal((D, PS)) / math.sqrt(D)).astype(np.float32),
        "b_head": np.zeros(PS, np.float32),
    }
    got = kernel(**ins)
    print("kernel loss:", got)
